# revision 8
# baseline (speedup 1.0000x reference)
"""Distributed causal attention (qkv proj + RoPE + SDPA + out proj) on 8 trn2 cores.

Sharding: data-parallel over batch (B=2), tensor-parallel over heads
(12 heads -> 4 groups of 3). Core c handles batch c//4, heads 3*(c%4)..3*(c%4)+2.
Each core computes a partial output x_b @ Wqkv_heads -> attention -> @ Wo_rows;
the host sums the 4 head-group partials per batch (bf16 partials, fp32 sum).

v2 design notes (vs the 149us baseline):
- PE warm-up matmul stream at t=0 so the HAM clock gate is released before
  the first projection matmul.
- DMA order: wq/xts-h0 interleaved, cos/sin h0 right behind, h1 tensors last;
  projection+RoPE proceed per T-half so unit (01,0)'s exp starts ~15us in
  (was 40.5us).  The h1 projections and late v-projections are injected into
  the attention-unit pipeline as filler steps (allocating from the wo psum
  pool).
- RoPE rotate-half row swap done by 4 SBUF->SBUF DMAs (sync engine) instead
  of 4 quarter-height DVE multiplies; DVE then does mul/mul/add at full 128
  partitions.
- Causal fine-trim: for diagonal chunk c = 4j+u only queries q' >= 128u are
  computed (scores matmul N, exp N, PV matmul N all trimmed); the remaining
  triangle is masked with a single [128,128] multiply per (head, chunk).
- ACT runs exp only during the attention phase (proj copies happen while ACT
  is otherwise idle; late v-proj + wo copies moved to DVE; the final unit's
  wo copies return to ACT, which is idle after the last exp).
- Softmax denominators: reciprocal_approx_fast reads the PV psum row
  directly (no denom staging copy); partition-broadcast still via the DRAM
  bounce.
- Output written bf16 (halves the out DMA); host accumulates in fp32.
"""
import numpy as np

B, T, C = 2, 2048, 768
H, DH = 12, 64
HPC = 3            # heads per core
NC_ = 8            # cores
QB = 512           # query block
KC = 128           # key chunk
HF = T // 2        # T-half for the projection pipeline
NJ = T // QB       # 4 query blocks
NKC = T // KC      # 16 key chunks
SCALE = 1.0 / float(np.sqrt(DH))

_prog = None


def _build():
    import concourse.bass as bass
    import concourse.tile as tile
    from concourse import bacc, mybir

    f32 = mybir.dt.float32
    bf16 = mybir.dt.bfloat16
    Exp = mybir.ActivationFunctionType.Exp

    nc = bacc.Bacc("TRN2", target_bir_lowering=False, debug=False)

    xT_p = nc.declare_dram_parameter("xT", [C, T], bf16, isOutput=False)
    wqkv_p = nc.declare_dram_parameter("wqkv", [C, 576], bf16, isOutput=False)
    wo_p = nc.declare_dram_parameter("wo", [HPC * DH, C], bf16, isOutput=False)
    cos_p = nc.declare_dram_parameter("cosT", [128, T], bf16, isOutput=False)
    sin_p = nc.declare_dram_parameter("sinT", [128, T], bf16, isOutput=False)
    out_p = nc.declare_dram_parameter("out", [T, C], bf16, isOutput=True)
    # DRAM bounce for the softmax-reciprocal partition-broadcast (SBUF APs
    # cannot have a zero partition step; DRAM APs can)
    recd_d = nc.dram_tensor("recd_dram", [1, HPC * QB], f32)

    with tile.TileContext(nc) as tc:
        with tc.tile_pool(name="persist", bufs=1) as persist:
            q01 = persist.tile([128, T], bf16, tag="q01")
            k01 = persist.tile([128, T], bf16, tag="k01")
            qk2 = persist.tile([128, T], bf16, tag="qk2")   # rows 0:64 q2, 64:128 dup
            k2al = persist.tile([128, T], bf16, tag="k2al")  # rows 0:64 k2, 64:128 dup
            xm2 = persist.tile([128, HF], bf16, tag="xm2")   # m2 proj staging
            vones = persist.tile([128, NKC, HPC, DH + 1], bf16, tag="vones")
            tri = persist.tile([128, KC], bf16, tag="tri")
            tri2 = persist.tile([128, 2 * KC], bf16, tag="tri2")
            warm = persist.tile([1, 16], f32, tag="warm")
            wpe = persist.tile([1, 16], bf16, tag="wpe")
            wq = persist.tile([128, 6, 576], bf16, tag="wq")
            xts = [persist.tile([128, T], bf16, tag=f"xt{k}", name=f"xt{k}")
                   for k in range(6)]
            cosT = persist.tile([128, T], bf16, tag="cosT")
            sinT = persist.tile([128, T], bf16, tag="sinT")

            # preload the exp table set while the input DMAs run
            nc.vector.memset(warm, 0.0)
            nc.vector.memset(wpe, 0.0)
            nc.scalar.activation(out=warm[0:1, 0:8], in_=warm[0:1, 0:8],
                                 func=Exp, scale=1.0)
            # causal triangle masks: tri[k, q'] = 1 if q' >= k else 0
            nc.gpsimd.memset(tri, 1.0)
            nc.gpsimd.affine_select(
                out=tri, in_=tri,
                compare_op=mybir.AluOpType.is_ge, fill=0.0, base=0,
                pattern=[[1, KC]], channel_multiplier=-1,
            )
            # tri2 = [zeros | tri] for the head-2 diagonal pair mask
            nc.gpsimd.memset(tri2[:, 0:KC], 0.0)
            nc.gpsimd.memset(tri2[:, KC:2 * KC], 1.0)
            nc.gpsimd.affine_select(
                out=tri2[:, KC:2 * KC], in_=tri2[:, KC:2 * KC],
                compare_op=mybir.AluOpType.is_ge, fill=0.0, base=0,
                pattern=[[1, KC]], channel_multiplier=-1,
            )
            # ones column of vones (for the fused softmax denominator)
            nc.gpsimd.memset(vones[:, :, :, DH:DH + 1], 1.0)

            h0 = slice(0, HF)
            h1 = slice(HF, T)

            with tc.tile_pool(name="rp", bufs=2) as rp:

                def rope(X, sl, out_q=None, out_k=None):
                    """RoPE X[:, sl] in place (X=q01/k01), or X=xm2 (local
                    cols 0:w) into out_q/out_k rows 0:64.  Row swap via
                    SBUF->SBUF DMA, sign folded into sinT."""
                    w = sl.stop - sl.start
                    tp = rp.tile([128, HF], bf16, tag="tp")
                    lsl = slice(0, w) if out_k is not None else sl
                    for r in (0, 64):
                        nc.sync.dma_start(out=tp[r:r + 32, 0:w],
                                          in_=X[r + 32:r + 64, lsl])
                        nc.sync.dma_start(out=tp[r + 32:r + 64, 0:w],
                                          in_=X[r:r + 32, lsl])
                    nc.vector.tensor_mul(tp[:, 0:w], tp[:, 0:w], sinT[:, sl])
                    if out_k is None:
                        nc.vector.tensor_mul(X[:, sl], X[:, sl], cosT[:, sl])
                        nc.vector.tensor_add(X[:, sl], X[:, sl], tp[:, 0:w])
                    else:
                        # cos product into a full-height scratch so the adds
                        # have base-partition-aligned inputs (DVE requires
                        # in0.base == in1.base)
                        ct = rp.tile([128, HF], bf16, tag="tp")
                        nc.vector.tensor_mul(ct[:, 0:w], X[:, lsl],
                                             cosT[:, sl])
                        nc.vector.tensor_add(out_q[0:64, sl], ct[0:64, 0:w],
                                             tp[0:64, 0:w])
                        nc.vector.tensor_add(out_k[0:64, sl], ct[64:128, 0:w],
                                             tp[64:128, 0:w])

                def proj_qk(m, half, pool, on_act):
                    """qkvT M-tile m for one T-half: two N=512 psum tiles
                    (matmul output must stay within one bank) + copy + RoPE."""
                    hsl = h0 if half == 0 else h1
                    cp = nc.scalar.copy if on_act else nc.vector.tensor_copy
                    X = q01 if m == 0 else (k01 if m == 1 else xm2)
                    for nn in range(2):
                        ps = pool.tile([128, QB], f32, tag=pool.name,
                                       name=f"pp{m}_{half}_{nn}")
                        for k in range(6):
                            nc.tensor.matmul(
                                ps,
                                lhsT=wq[:, k, m * 128:(m + 1) * 128],
                                rhs=xts[k][:, half * HF + nn * QB:
                                           half * HF + (nn + 1) * QB],
                                start=(k == 0), stop=(k == 5))
                        if m == 2:
                            cp(xm2[:, nn * QB:(nn + 1) * QB], ps)
                        else:
                            cp(X[:, half * HF + nn * QB:
                                 half * HF + (nn + 1) * QB], ps)
                    if m == 0:
                        rope(q01, hsl)
                    elif m == 1:
                        rope(k01, hsl)
                    else:
                        rope(xm2, hsl, out_q=qk2, out_k=k2al)
                        # duplicate rows 64:128 so head-2 matmuls can
                        # alternate PE row halves
                        nc.sync.dma_start(out=qk2[64:128, hsl],
                                          in_=qk2[0:64, hsl])
                        nc.sync.dma_start(out=k2al[64:128, hsl],
                                          in_=k2al[0:64, hsl])

                def vproj(t, pool, on_act):
                    ps = pool.tile([128, 192], f32, tag=pool.name,
                                   name=f"vps{t}")
                    for k in range(6):
                        nc.tensor.matmul(
                            ps, lhsT=xts[k][:, t * 128:(t + 1) * 128],
                            rhs=wq[:, k, 384:576],
                            start=(k == 0), stop=(k == 5))
                    cp = nc.scalar.copy if on_act else nc.vector.tensor_copy
                    cp(vones[:, t, :, 0:DH],
                       ps.rearrange("p (h d) -> p h d", h=HPC))

                with tc.tile_pool(name="pp", bufs=2, space="PSUM") as pp, \
                     tc.tile_pool(name="vp", bufs=2, space="PSUM") as vp, \
                     tc.tile_pool(name="wmp", bufs=1, space="PSUM") as wmp:
                    # PE warm-up: tiny matmuls fill the DMA wait and release
                    # the HAM throttle before the first projection matmul
                    wps = wmp.tile([1, 16], f32, tag="wps")
                    for _ in range(80):
                        nc.tensor.matmul(wps, lhsT=wpe[0:1, 0:1],
                                         rhs=wpe[0:1, :],
                                         start=True, stop=True)

                    # input DMAs, ordered by first use: xts-h0 + the q/k
                    # weight slices, rope tables h0, the m2/v weight slices,
                    # then everything h1
                    for k in range(6):
                        nc.sync.dma_start(out=xts[k][:, h0],
                                          in_=xT_p[k * 128:(k + 1) * 128, h0])
                        nc.sync.dma_start(
                            out=wq[:, k, 0:256],
                            in_=wqkv_p[k * 128:(k + 1) * 128, 0:256])
                    nc.sync.dma_start(out=cosT[:, h0], in_=cos_p[:, h0])
                    nc.sync.dma_start(out=sinT[:, h0], in_=sin_p[:, h0])
                    for k in range(6):
                        nc.sync.dma_start(
                            out=wq[:, k, 256:576],
                            in_=wqkv_p[k * 128:(k + 1) * 128, 256:576])
                    for k in range(6):
                        nc.sync.dma_start(out=xts[k][:, h1],
                                          in_=xT_p[k * 128:(k + 1) * 128, h1])
                    nc.sync.dma_start(out=cosT[:, h1], in_=cos_p[:, h1])
                    nc.sync.dma_start(out=sinT[:, h1], in_=sin_p[:, h1])

                    # critical path to the first attention unit: h0 proj
                    proj_qk(0, 0, pp, on_act=True)
                    proj_qk(1, 0, pp, on_act=True)
                    for t in range(4):
                        vproj(t, vp, on_act=True)
                    proj_qk(2, 0, pp, on_act=True)

                # --- attention + per-block output projection ---
                with tc.tile_pool(name="phaseB", bufs=1) as pb, \
                     tc.tile_pool(name="bct", bufs=2) as bcp, \
                     tc.tile_pool(name="ostage", bufs=3) as osp:
                    expts = [pb.tile([128, 2, NKC, QB], bf16,
                                     name=f"expt{i}", tag=f"expt{i}")
                             for i in range(2)]
                    outt01 = pb.tile([128, T], bf16, tag="outt01")
                    outt2 = pb.tile([64, T], bf16, tag="outt2")
                    recd = pb.tile([1, HPC * QB], f32, tag="recd")
                    dnm = pb.tile([1, HPC * QB], f32, tag="dnm")
                    wo01 = pb.tile([128, C], bf16, tag="wo01")
                    nc.sync.dma_start(out=wo01, in_=wo_p[0:128, :])
                    wo2 = pb.tile([64, C], bf16, tag="wo2")
                    nc.sync.dma_start(out=wo2, in_=wo_p[128:192, :])

                    def tgt_of(h):
                        return (outt01[0:64] if h == 0 else
                                (outt01[64:128] if h == 1 else outt2[0:64]))

                    with tc.tile_pool(name="sc", bufs=2, space="PSUM") as scp, \
                         tc.tile_pool(name="pv", bufs=2, space="PSUM") as pvp, \
                         tc.tile_pool(name="wp", bufs=1, space="PSUM") as wpp:

                        def s_steps(unit, expt):
                            """Score-group closures: matmuls + exp (+ causal
                            triangle mask on diagonal chunks), causal-trimmed
                            to q' >= 128u for diagonal chunk 4j+u."""
                            hh, j = unit
                            steps = []
                            if hh == "01":
                                def grp01(c):
                                    u = c - 4 * j
                                    off = KC * u if u > 0 else 0
                                    qsl = slice(j * QB + off, (j + 1) * QB)
                                    sc = scp.tile([128, 2, QB], f32, tag="sc",
                                                  name=f"sc01_{j}_{c}")
                                    nc.tensor.matmul(
                                        sc[:, 0, off:QB],
                                        lhsT=k01[0:64, c * KC:(c + 1) * KC],
                                        rhs=q01[0:64, qsl],
                                        start=True, stop=True)
                                    nc.tensor.matmul(
                                        sc[:, 1, off:QB],
                                        lhsT=k01[64:128, c * KC:(c + 1) * KC],
                                        rhs=q01[64:128, qsl],
                                        start=True, stop=True)
                                    nc.scalar.activation(
                                        out=expt[:, :, c, off:QB],
                                        in_=sc[:, :, off:QB],
                                        func=Exp, scale=SCALE)
                                    if u >= 0:
                                        for hh_ in range(2):
                                            nc.vector.tensor_mul(
                                                expt[:, hh_, c, off:off + KC],
                                                expt[:, hh_, c, off:off + KC],
                                                tri)
                                for c in range(4 * (j + 1)):
                                    steps.append(lambda c=c: grp01(c))
                            else:
                                def grp2(g):
                                    c0 = 2 * g
                                    u0 = c0 - 4 * j
                                    off = KC * u0 if u0 > 0 else 0
                                    qsl = slice(j * QB + off, (j + 1) * QB)
                                    sc = scp.tile([128, 2, QB], f32, tag="sc",
                                                  name=f"sc2_{j}_{g}")
                                    for uu in range(2):
                                        c = c0 + uu
                                        lo = c % 2 == 0
                                        kk = k2al[0:64] if lo else k2al[64:128]
                                        qq = qk2[0:64] if lo else qk2[64:128]
                                        nc.tensor.matmul(
                                            sc[:, uu, off:QB],
                                            lhsT=kk[:, c * KC:(c + 1) * KC],
                                            rhs=qq[:, qsl],
                                            start=True, stop=True)
                                    nc.scalar.activation(
                                        out=expt[:, 0, c0:c0 + 2, off:QB],
                                        in_=sc[:, :, off:QB],
                                        func=Exp, scale=SCALE)
                                    if u0 >= 0:
                                        # chunk c0: triangle at [off,off+128);
                                        # c0+1: zero+triangle [off,off+256)
                                        nc.vector.tensor_mul(
                                            expt[:, 0, c0, off:off + KC],
                                            expt[:, 0, c0, off:off + KC],
                                            tri)
                                        nc.vector.tensor_mul(
                                            expt[:, 0, c0 + 1,
                                                 off:off + 2 * KC],
                                            expt[:, 0, c0 + 1,
                                                 off:off + 2 * KC],
                                            tri2)
                                for g in range(2 * (j + 1)):
                                    steps.append(lambda g=g: grp2(g))
                            return steps

                        def p_steps(unit, expt, last=False):
                            """PV matmul chunk-steps, then copy+reciprocal,
                            then (after the '2' unit) the block's Wo."""
                            hh, j = unit
                            nch = 4 * (j + 1)
                            heads = [(0, 0), (1, 1)] if hh == "01" else [(2, 0)]
                            pos = {}
                            steps = []

                            def setup():
                                for h, _ in heads:
                                    pos[h] = pvp.tile([128, QB], f32,
                                                      tag="pv",
                                                      name=f"po_{h}_{j}")

                            def chunk(c):
                                u = c - 4 * j
                                off = KC * u if u > 0 else 0
                                for h, hh_slot in heads:
                                    nc.tensor.matmul(
                                        pos[h][0:DH + 1, off:QB],
                                        lhsT=vones[:, c, h, :],
                                        rhs=expt[:, hh_slot, c, off:QB],
                                        start=(c == 0), stop=(c == nch - 1),
                                        skip_group_check=True)

                            steps.append(setup)
                            for c0 in range(0, nch, 2):
                                def two(c0=c0):
                                    chunk(c0)
                                    chunk(c0 + 1)
                                steps.append(two)

                            def fin(h, hh_slot):
                                po = pos[h]
                                nc.vector.tensor_copy(
                                    tgt_of(h)[:, j * QB:(j + 1) * QB],
                                    po[0:DH, :])
                                nc.vector.tensor_copy(
                                    dnm[0:1, h * QB:(h + 1) * QB],
                                    po[DH:DH + 1, :])

                            def norm_unit():
                                usl = slice(heads[0][0] * QB,
                                            (heads[-1][0] + 1) * QB)
                                with nc.allow_low_precision(
                                        reason="softmax denom reciprocal"):
                                    nc.vector.reciprocal_approx_fast(
                                        out=recd[0:1, usl],
                                        in_=dnm[0:1, usl])
                                nc.sync.dma_start(out=recd_d[0:1, usl],
                                                  in_=recd[0:1, usl])
                                for h, _ in heads:
                                    base = 64 if h == 1 else 0
                                    src = recd_d[0:1, h * QB:(h + 1) * QB]
                                    bsrc = bass.AP(
                                        tensor=src.tensor, offset=src.offset,
                                        ap=[[0, 64]] + list(src.ap[1:]))
                                    bct = bcp.tile([128, QB], f32, tag="bct",
                                                   name=f"bct_{h}_{j}")
                                    nc.sync.dma_start(
                                        out=bct[base:base + 64, :], in_=bsrc)
                                    tgt = tgt_of(h)
                                    sl = slice(j * QB, (j + 1) * QB)
                                    nc.vector.tensor_mul(
                                        tgt[:, sl], tgt[:, sl],
                                        bct[base:base + 64, :])

                            for h, hh_slot in heads:
                                steps.append(lambda h=h, s=hh_slot: fin(h, s))
                            steps.append(norm_unit)

                            if hh == "2":
                                pws = {}

                                def wo_mm(qq):
                                    q = j * 4 + qq
                                    pwA = wpp.tile([128, QB], f32, tag="wp",
                                                   name=f"pwA_{q}")
                                    pwB = wpp.tile([128, 256], f32, tag="wp",
                                                   name=f"pwB_{q}")
                                    pws[qq] = (pwA, pwB)
                                    for pw, (n0, n1) in ((pwA, (0, 512)),
                                                         (pwB, (512, 768))):
                                        nc.tensor.matmul(
                                            pw,
                                            lhsT=outt01[:,
                                                        q * 128:(q + 1) * 128],
                                            rhs=wo01[:, n0:n1],
                                            start=True, stop=False)
                                        nc.tensor.matmul(
                                            pw,
                                            lhsT=outt2[:,
                                                       q * 128:(q + 1) * 128],
                                            rhs=wo2[:, n0:n1],
                                            start=False, stop=True)

                                def wo_out(qq):
                                    q = j * 4 + qq
                                    pwA, pwB = pws[qq]
                                    ot = osp.tile([128, C], bf16, tag="ot",
                                                  name=f"ot_{q}")
                                    cpf = (nc.scalar.copy if last
                                           else nc.vector.tensor_copy)
                                    cpf(ot[:, 0:QB], pwA)
                                    cpf(ot[:, QB:C], pwB)
                                    nc.sync.dma_start(
                                        out=out_p[q * 128:(q + 1) * 128, :],
                                        in_=ot)
                                for qq in range(4):
                                    steps.append(lambda qq=qq: wo_mm(qq))
                                    steps.append(lambda qq=qq: wo_out(qq))
                            return steps

                        def vproj_steps(ts):
                            return [lambda t=t: vproj(t, wpp, on_act=False)
                                    for t in ts]

                        def proj_steps(m):
                            return [lambda m=m: proj_qk(m, 1, wpp,
                                                        on_act=False)]

                        units = [("01", 0), ("2", 0), ("01", 1), ("2", 1),
                                 ("01", 2), ("2", 2), ("01", 3), ("2", 3)]
                        # filler work injected after unit i completes (so it
                        # executes interleaved under unit i+1's exp):
                        fillers = {
                            0: proj_steps(0),
                            1: proj_steps(1) + vproj_steps(range(4, 8)),
                            3: proj_steps(2) + vproj_steps(range(8, 12)),
                            4: vproj_steps(range(12, 16)),
                        }

                        # lag-1 pipeline, interleaved at step granularity: PE
                        # runs the previous unit's PV/Wo steps in the gaps
                        # between this unit's score groups (paced by ACT exp).
                        prev_p = []
                        for i, u in enumerate(units):
                            last = i == len(units) - 1
                            S = s_steps(u, expts[i % 2])
                            if last:
                                # fold the final unit's own PV behind its
                                # score groups (lag 2): own chunk-step k
                                # needs exp group k.
                                own = p_steps(u, expts[i % 2], last=True)
                            done = 0
                            own_done = 0
                            for gi, s in enumerate(S):
                                s()
                                want = ((gi + 1) * len(prev_p)) // len(S)
                                while done < want:
                                    prev_p[done]()
                                    done += 1
                                if last and gi >= 2:
                                    while own_done < min(gi - 1,
                                                         len(S) - 1) + 1:
                                        own[own_done]()
                                        own_done += 1
                            while done < len(prev_p):
                                prev_p[done]()
                                done += 1
                            if last:
                                prev_p = own[own_done:]
                            else:
                                prev_p = p_steps(u, expts[i % 2])
                                prev_p = prev_p + fillers.get(i, [])
                        for p in prev_p:
                            p()

    nc.compile()
    return nc


def _host_prep(x, Wqkv, Wo, seq_len):
    import ml_dtypes
    bf16 = ml_dtypes.bfloat16
    x = np.asarray(x, dtype=np.float32)
    Wqkv = np.asarray(Wqkv, dtype=np.float32)
    Wo = np.asarray(Wo, dtype=np.float32)
    off = int(np.asarray(seq_len).reshape(()))

    inv = 1.0 / (10000.0 ** (np.arange(0, DH, 2, dtype=np.float64) / DH))  # [32]
    pos = np.arange(T, dtype=np.float64) + off
    ang = pos[:, None] * inv[None, :]                 # [T, 32]
    cs = np.cos(ang).T                                # [32, T]
    sn = np.sin(ang).T
    cos128 = np.empty((128, T), np.float32)
    sin128 = np.empty((128, T), np.float32)
    for blk in range(2):
        r0 = blk * 64
        cos128[r0:r0 + 32] = cs
        cos128[r0 + 32:r0 + 64] = cs
        # sign-folded (NOT row-swapped: the row swap happens on-device via
        # SBUF->SBUF DMA): row r multiplies the swapped operand X[sigma(r)],
        # with sign -1 for the first half of each 64-row block.
        sin128[r0:r0 + 32] = -sn
        sin128[r0 + 32:r0 + 64] = sn

    in_maps = []
    for core in range(NC_):
        b, g = core // 4, core % 4
        hs = [3 * g, 3 * g + 1, 3 * g + 2]
        q = [Wqkv[:, h * DH:(h + 1) * DH] for h in hs]
        k = [Wqkv[:, C + h * DH:C + (h + 1) * DH] for h in hs]
        v = [Wqkv[:, 2 * C + h * DH:2 * C + (h + 1) * DH] for h in hs]
        wqkv_l = np.concatenate(
            [q[0], q[1], k[0], k[1], q[2], k[2], v[0], v[1], v[2]], axis=1)
        in_maps.append({
            "xT": np.ascontiguousarray(x[b].T).astype(bf16),
            "wqkv": np.ascontiguousarray(wqkv_l).astype(bf16),
            "wo": np.ascontiguousarray(
                Wo[g * HPC * DH:(g + 1) * HPC * DH, :]).astype(bf16),
            "cosT": cos128.astype(bf16),
            "sinT": sin128.astype(bf16),
        })
    return in_maps


def _run(in_maps, trace=False):
    global _prog
    from concourse.bass_utils import run_bass_kernel_spmd
    if _prog is None:
        _prog = _build()
    return run_bass_kernel_spmd(_prog, in_maps, list(range(NC_)), trace=trace)


def kernel(x, Wqkv, Wo, seq_len):
    in_maps = _host_prep(x, Wqkv, Wo, seq_len)
    res = _run(in_maps, trace=False)
    out = np.zeros((B, T, C), dtype=np.float32)
    for core in range(NC_):
        out[core // 4] += res.results[core]["out"].astype(np.float32)
    return out


# revision 10
# speedup vs baseline: 1.4448x; 1.4448x over previous
"""Distributed causal attention (qkv proj + RoPE + SDPA + out proj) on 8 trn2 cores.

Sharding: data-parallel over batch (B=2), tensor-parallel over heads
(12 heads -> 4 groups of 3). Core c handles batch c//4, heads 3*(c%4)..3*(c%4)+2.
Each core computes a partial output x_b @ Wqkv_heads -> attention -> @ Wo_rows;
the host sums the 4 head-group partials per batch (bf16 partials, fp32 sum).

Device layout per core (bf16 matmul operands, fp32 PSUM accumulation):
  xT    [768, 2048]  x[b] transposed (C-major), bf16
  wqkv  [768, 576]   columns: [q0 q1 | k0 k1 | q2 k2 | v0 v1 v2] (64 each), bf16
  wo    [192, 768]   Wo rows for the 3 heads, bf16
  cosT/sinT [128, 2048] RoPE tables, bf16 (sinT sign-folded; the rotate-half
  row swap happens on device via SBUF->SBUF DMA)
  out   [2048, 768]  bf16 partial (pre-reduction) output

Attention is a lag-1 software pipeline over units
  [(01,0), (01,1), (2,0), (01,2), (2,1), (01,3), (2,2), (2,3)]:
PE runs the scores matmuls of unit i+1 while ACT exponentiates unit i, then PE's
PV matmuls of unit i follow.  Unit (01,j) computes heads 0 and 1 together with
row-packed K=64 matmuls; unit (2,j) does head 2 alone, alternating row halves
via duplicated q2/k2.  After both units of query-block j are normalized, that
block's output projection + DMA-out run.

vs the 149us baseline: causal fine-trim at 128-query granularity (scores/exp/
PV only touch q' >= 128u of diagonal chunk 4j+u; [128,128] triangle masks),
RoPE row-swap via SBUF->SBUF DMA, PE clock-gate warm-up, rope tables DMAed
per T-half right after the h0 inputs, exp-only ACT during attention, bf16
output partials.
"""
import numpy as np

B, T, C = 2, 2048, 768
H, DH = 12, 64
HPC = 3            # heads per core
NC_ = 8            # cores
QB = 512           # query block
KC = 128           # key chunk
HF = T // 2
NJ = T // QB       # 4 query blocks
NKC = T // KC      # 16 key chunks
SCALE = 1.0 / float(np.sqrt(DH))

_prog = None


def _build():
    import concourse.bass as bass
    import concourse.tile as tile
    from concourse import bacc, mybir

    f32 = mybir.dt.float32
    bf16 = mybir.dt.bfloat16
    Exp = mybir.ActivationFunctionType.Exp

    nc = bacc.Bacc("TRN2", target_bir_lowering=False, debug=False)

    xT_p = nc.declare_dram_parameter("xT", [C, T], bf16, isOutput=False)
    wqkv_p = nc.declare_dram_parameter("wqkv", [C, 576], bf16, isOutput=False)
    wo_p = nc.declare_dram_parameter("wo", [HPC * DH, C], bf16, isOutput=False)
    cos_p = nc.declare_dram_parameter("cosT", [128, T], bf16, isOutput=False)
    sin_p = nc.declare_dram_parameter("sinT", [128, T], bf16, isOutput=False)
    out_p = nc.declare_dram_parameter("out", [T, C], bf16, isOutput=True)
    # DRAM bounce for the softmax-reciprocal partition-broadcast (SBUF APs
    # cannot have a zero partition step; DRAM APs can)
    recd_d = nc.dram_tensor("recd_dram", [1, HPC * QB], f32)

    with tile.TileContext(nc) as tc:
        with tc.tile_pool(name="persist", bufs=1) as persist:
            q01 = persist.tile([128, T], bf16, tag="q01")
            k01 = persist.tile([128, T], bf16, tag="k01")
            qk2 = persist.tile([128, T], bf16, tag="qk2")   # rows 0:64 q2, 64:128 q2 dup
            k2al = persist.tile([128, T], bf16, tag="k2al")  # rows 0:64 k2, 64:128 k2 dup
            vones = persist.tile([128, NKC, HPC, DH + 1], bf16, tag="vones")
            tri = persist.tile([128, KC], bf16, tag="tri")
            tri2 = persist.tile([128, 2 * KC], bf16, tag="tri2")
            warm = persist.tile([1, 16], f32, tag="warm")
            wpe = persist.tile([1, 16], bf16, tag="wpe")
            wq = persist.tile([128, 6, 576], bf16, tag="wq")
            xts = [persist.tile([128, T], bf16, tag=f"xt{k}", name=f"xt{k}")
                   for k in range(6)]
            cosT = persist.tile([128, T], bf16, tag="cosT")
            sinT = persist.tile([128, T], bf16, tag="sinT")

            # preload the exp table set while DMAs run
            nc.vector.memset(warm, 0.0)
            nc.vector.memset(wpe, 0.0)
            nc.scalar.activation(out=warm[0:1, 0:8], in_=warm[0:1, 0:8],
                                 func=Exp, scale=1.0)
            # causal triangle masks: tri[k, q'] = 1 if q' >= k else 0
            nc.gpsimd.memset(tri, 1.0)
            nc.gpsimd.affine_select(
                out=tri, in_=tri,
                compare_op=mybir.AluOpType.is_ge, fill=0.0, base=0,
                pattern=[[1, KC]], channel_multiplier=-1,
            )
            # tri2 = [zeros | tri] for the head-2 diagonal pair mask
            nc.gpsimd.memset(tri2[:, 0:KC], 0.0)
            nc.gpsimd.memset(tri2[:, KC:2 * KC], 1.0)
            nc.gpsimd.affine_select(
                out=tri2[:, KC:2 * KC], in_=tri2[:, KC:2 * KC],
                compare_op=mybir.AluOpType.is_ge, fill=0.0, base=0,
                pattern=[[1, KC]], channel_multiplier=-1,
            )
            # ones column of vones (for the fused softmax denominator)
            nc.gpsimd.memset(vones[:, :, :, DH:DH + 1], 1.0)

            h0 = slice(0, HF)
            h1 = slice(HF, T)

            with tc.tile_pool(name="pp", bufs=1, space="PSUM") as pp, \
                 tc.tile_pool(name="vp", bufs=2, space="PSUM") as vp, \
                 tc.tile_pool(name="wmp", bufs=1, space="PSUM") as wmp, \
                 tc.tile_pool(name="rp", bufs=2) as rp:
                # PE warm-up: tiny matmuls fill the DMA wait and release the
                # HAM clock throttle before the first projection matmul
                wps = wmp.tile([1, 16], f32, tag="wps")
                for _ in range(80):
                    nc.tensor.matmul(wps, lhsT=wpe[0:1, 0:1], rhs=wpe[0:1, :],
                                     start=True, stop=True)

                # input DMAs, in first-use order: h0 of everything, then h1
                for k in range(6):
                    nc.sync.dma_start(
                        out=wq[:, k, :], in_=wqkv_p[k * 128:(k + 1) * 128, :])
                    nc.sync.dma_start(out=xts[k][:, h0],
                                      in_=xT_p[k * 128:(k + 1) * 128, h0])
                nc.sync.dma_start(out=cosT[:, h0], in_=cos_p[:, h0])
                nc.sync.dma_start(out=sinT[:, h0], in_=sin_p[:, h0])
                for k in range(6):
                    nc.sync.dma_start(out=xts[k][:, h1],
                                      in_=xT_p[k * 128:(k + 1) * 128, h1])
                nc.sync.dma_start(out=cosT[:, h1], in_=cos_p[:, h1])
                nc.sync.dma_start(out=sinT[:, h1], in_=sin_p[:, h1])

                def rope(X, out_q, out_k, sl):
                    """RoPE X[:, sl] in place (or, for the q2k2 tile, into
                    out_q/out_k rows 0:64).  Rotate-half row swap via 4
                    SBUF->SBUF DMAs; sin sign-folding is in the host table."""
                    w = sl.stop - sl.start
                    tp = rp.tile([128, HF], bf16, tag="tp")
                    for r in (0, 64):
                        nc.sync.dma_start(out=tp[r:r + 32, 0:w],
                                          in_=X[r + 32:r + 64, sl])
                        nc.sync.dma_start(out=tp[r + 32:r + 64, 0:w],
                                          in_=X[r:r + 32, sl])
                    nc.vector.tensor_mul(tp[:, 0:w], tp[:, 0:w], sinT[:, sl])
                    if out_k is None:
                        nc.vector.tensor_mul(X[:, sl], X[:, sl], cosT[:, sl])
                        nc.vector.tensor_add(X[:, sl], X[:, sl], tp[:, 0:w])
                    else:
                        # cos product into a full-height scratch so the adds
                        # have base-partition-aligned inputs
                        ct = rp.tile([128, HF], bf16, tag="tp")
                        nc.vector.tensor_mul(ct[:, 0:w], X[:, sl],
                                             cosT[:, sl])
                        nc.vector.tensor_add(out_q[0:64, sl], ct[0:64, 0:w],
                                             tp[0:64, 0:w])
                        nc.vector.tensor_add(out_k[0:64, sl], ct[64:128, 0:w],
                                             tp[64:128, 0:w])

                # q/k projection: M-tile m of qkvT = wqkv cols [128m, 128m+128)
                def proj_qk(m, X, out_q=None, out_k=None, half=0):
                    pst = []
                    for nn in range(2):
                        ps = pp.tile([128, QB], f32, tag=f"pp{nn}",
                                     name=f"pp{m}_{half}_{nn}")
                        pst.append(ps)
                    for k in range(6):
                        for nn in range(2):
                            n = half * 2 + nn
                            nc.tensor.matmul(
                                pst[nn],
                                lhsT=wq[:, k, m * 128:(m + 1) * 128],
                                rhs=xts[k][:, n * QB:(n + 1) * QB],
                                start=(k == 0), stop=(k == 5))
                    for nn in range(2):
                        n = half * 2 + nn
                        nc.scalar.copy(X[:, n * QB:(n + 1) * QB], pst[nn])
                    rope(X, out_q, out_k,
                         slice(half * HF, (half + 1) * HF))

                def vproj(t, on_act, pool, tag="vp"):
                    ps = pool.tile([128, 192], f32, tag=tag, name=f"vps{t}")
                    for k in range(6):
                        nc.tensor.matmul(
                            ps, lhsT=xts[k][:, t * 128:(t + 1) * 128],
                            rhs=wq[:, k, 384:576],
                            start=(k == 0), stop=(k == 5))
                    cp = nc.scalar.copy if on_act else nc.vector.tensor_copy
                    cp(vones[:, t, :, 0:DH],
                       ps.rearrange("p (h d) -> p h d", h=HPC))

                # h0 of q01+k01 first (unblocks the first score units), then
                # the v chunks they need, then everything else
                proj_qk(0, q01, half=0)
                proj_qk(1, k01, half=0)
                for t in range(4):
                    vproj(t, True, vp)
                proj_qk(2, qk2, out_q=qk2, out_k=k2al, half=0)
                nc.sync.dma_start(out=qk2[64:128, h0], in_=qk2[0:64, h0])
                nc.sync.dma_start(out=k2al[64:128, h0], in_=k2al[0:64, h0])
                proj_qk(0, q01, half=1)
                proj_qk(1, k01, half=1)
                proj_qk(2, qk2, out_q=qk2, out_k=k2al, half=1)
                nc.sync.dma_start(out=qk2[64:128, h1], in_=qk2[0:64, h1])
                nc.sync.dma_start(out=k2al[64:128, h1], in_=k2al[0:64, h1])

            # --- attention + per-block output projection ---
            with tc.tile_pool(name="phaseB", bufs=1) as pb, \
                 tc.tile_pool(name="bct", bufs=2) as bcp, \
                 tc.tile_pool(name="ostage", bufs=3) as osp:
                expts = [pb.tile([128, 2, NKC, QB], bf16, name=f"expt{i}",
                                 tag=f"expt{i}")
                         for i in range(2)]
                outt01 = pb.tile([128, T], bf16, tag="outt01")
                outt2 = pb.tile([64, T], bf16, tag="outt2")
                # denominators, row r = j*HPC + h; recd = 1/denom
                denom = pb.tile([1, HPC * QB], f32, tag="denom")
                recd = pb.tile([1, HPC * QB], f32, tag="recd")
                wo01 = pb.tile([128, C], bf16, tag="wo01")
                nc.sync.dma_start(out=wo01, in_=wo_p[0:128, :])
                wo2 = pb.tile([64, C], bf16, tag="wo2")
                nc.sync.dma_start(out=wo2, in_=wo_p[128:192, :])

                def tgt_of(h):
                    return outt01[0:64] if h == 0 else (outt01[64:128] if h == 1 else outt2[0:64])

                with tc.tile_pool(name="sc", bufs=2, space="PSUM") as scp, \
                     tc.tile_pool(name="pv", bufs=2, space="PSUM") as pvp, \
                     tc.tile_pool(name="wp", bufs=1, space="PSUM") as wpp:

                    def s_steps(unit, expt):
                        """Closures: one per 2-matmul scores psum group (+exp,
                        +triangle masks on diagonal chunks).  Diagonal chunk
                        c=4j+u only computes queries q' >= 128u."""
                        hh, j = unit
                        steps = []
                        if hh == "01":
                            # heads 0+1 row-packed: per sc tile, 1 chunk each
                            def grp01(c):
                                u = c - 4 * j
                                off = KC * u if u > 0 else 0
                                qsl = slice(j * QB + off, (j + 1) * QB)
                                sc = scp.tile([128, 2, QB], f32, tag="sc",
                                              name=f"sc01_{j}_{c}")
                                nc.tensor.matmul(
                                    sc[:, 0, off:QB],
                                    lhsT=k01[0:64, c * KC:(c + 1) * KC],
                                    rhs=q01[0:64, qsl],
                                    start=True, stop=True)
                                nc.tensor.matmul(
                                    sc[:, 1, off:QB],
                                    lhsT=k01[64:128, c * KC:(c + 1) * KC],
                                    rhs=q01[64:128, qsl],
                                    start=True, stop=True)
                                nc.scalar.activation(
                                    out=expt[:, :, c, off:QB],
                                    in_=sc[:, :, off:QB],
                                    func=Exp, scale=SCALE)
                                if u >= 0:
                                    for hh_ in range(2):
                                        nc.vector.tensor_mul(
                                            expt[:, hh_, c, off:off + KC],
                                            expt[:, hh_, c, off:off + KC],
                                            tri)
                            for c in range(4 * (j + 1)):
                                steps.append(lambda c=c: grp01(c))
                        else:
                            # head 2: alternate row halves for LDW/MM overlap
                            def grp2(g):
                                c0 = 2 * g
                                u0 = c0 - 4 * j
                                off = KC * u0 if u0 > 0 else 0
                                qsl = slice(j * QB + off, (j + 1) * QB)
                                sc = scp.tile([128, 2, QB], f32, tag="sc",
                                              name=f"sc2_{j}_{g}")
                                for uu in range(2):
                                    c = c0 + uu
                                    lo = c % 2 == 0
                                    kk = k2al[0:64] if lo else k2al[64:128]
                                    qq = qk2[0:64] if lo else qk2[64:128]
                                    nc.tensor.matmul(
                                        sc[:, uu, off:QB],
                                        lhsT=kk[:, c * KC:(c + 1) * KC],
                                        rhs=qq[:, qsl],
                                        start=True, stop=True)
                                nc.scalar.activation(
                                    out=expt[:, 0, c0:c0 + 2, off:QB],
                                    in_=sc[:, :, off:QB],
                                    func=Exp, scale=SCALE)
                                if u0 >= 0:
                                    # c0: triangle; c0+1: zero+triangle
                                    nc.vector.tensor_mul(
                                        expt[:, 0, c0, off:off + KC],
                                        expt[:, 0, c0, off:off + KC],
                                        tri)
                                    nc.vector.tensor_mul(
                                        expt[:, 0, c0 + 1, off:off + 2 * KC],
                                        expt[:, 0, c0 + 1, off:off + 2 * KC],
                                        tri2)
                            for g in range(2 * (j + 1)):
                                steps.append(lambda g=g: grp2(g))
                        return steps

                    def p_steps(unit, expt, last=False):
                        """Closures: PV matmul chunk-steps, then copy+normalize,
                        then (after the '2' unit) the block's output projection."""
                        hh, j = unit
                        nch = 4 * (j + 1)
                        heads = [(0, 0), (1, 1)] if hh == "01" else [(2, 0)]
                        pos = {}
                        steps = []

                        def setup():
                            for h, _ in heads:
                                pos[h] = pvp.tile([128, QB], f32, tag="pv",
                                                  name=f"po_{h}_{j}")

                        def chunk(c):
                            u = c - 4 * j
                            off = KC * u if u > 0 else 0
                            for h, hh_slot in heads:
                                nc.tensor.matmul(
                                    pos[h][0:DH + 1, off:QB],
                                    lhsT=vones[:, c, h, :],
                                    rhs=expt[:, hh_slot, c, off:QB],
                                    start=(c == 0), stop=(c == nch - 1),
                                    skip_group_check=True)

                        steps.append(setup)
                        for c0 in range(0, nch, 2):
                            def two(c0=c0):
                                chunk(c0)
                                chunk(c0 + 1)
                            steps.append(two)

                        def fin(h, hh_slot):
                            po = pos[h]
                            nc.vector.tensor_copy(
                                tgt_of(h)[:, j * QB:(j + 1) * QB], po[0:DH, :])
                            nc.vector.tensor_copy(
                                denom[0:1, h * QB:(h + 1) * QB],
                                po[DH:DH + 1, :])

                        def norm_unit():
                            usl = slice(heads[0][0] * QB,
                                        (heads[-1][0] + 1) * QB)
                            with nc.allow_low_precision(reason="softmax denom reciprocal: 18-bit approx"):
                                nc.vector.reciprocal_approx_fast(
                                    out=recd[0:1, usl], in_=denom[0:1, usl])
                            nc.sync.dma_start(out=recd_d[0:1, usl],
                                              in_=recd[0:1, usl])
                            for h, _ in heads:
                                base = 64 if h == 1 else 0
                                src = recd_d[0:1, h * QB:(h + 1) * QB]
                                bsrc = bass.AP(
                                    tensor=src.tensor, offset=src.offset,
                                    ap=[[0, 64]] + list(src.ap[1:]))
                                bct = bcp.tile([128, QB], f32, tag="bct",
                                               name=f"bct_{h}_{j}")
                                nc.sync.dma_start(
                                    out=bct[base:base + 64, :], in_=bsrc)
                                tgt = tgt_of(h)
                                sl = slice(j * QB, (j + 1) * QB)
                                nc.vector.tensor_mul(
                                    tgt[:, sl], tgt[:, sl], bct[base:base + 64, :])

                        for h, hh_slot in heads:
                            steps.append(lambda h=h, s=hh_slot: fin(h, s))
                        steps.append(norm_unit)

                        if hh == "2":
                            pws = {}

                            def wo_mm(qq):
                                q = j * 4 + qq
                                pw = wpp.tile([128, 1024], f32, tag="wp",
                                              name=f"pw_{q}")
                                pws[qq] = pw
                                for (n0, n1) in ((0, 512), (512, 768)):
                                    nc.tensor.matmul(
                                        pw[:, n0:n1],
                                        lhsT=outt01[:, q * 128:(q + 1) * 128],
                                        rhs=wo01[:, n0:n1],
                                        start=True, stop=False)
                                    nc.tensor.matmul(
                                        pw[:, n0:n1],
                                        lhsT=outt2[:, q * 128:(q + 1) * 128],
                                        rhs=wo2[:, n0:n1],
                                        start=False, stop=True)

                            def wo_out(qq):
                                q = j * 4 + qq
                                pw = pws[qq]
                                ot = osp.tile([128, C], bf16, tag="ot",
                                              name=f"ot_{q}")
                                if last:
                                    nc.scalar.copy(ot, pw[:, 0:C])
                                else:
                                    nc.vector.tensor_copy(ot, pw[:, 0:C])
                                nc.sync.dma_start(
                                    out=out_p[q * 128:(q + 1) * 128, :], in_=ot)
                            for qq in range(4):
                                steps.append(lambda qq=qq: wo_mm(qq))
                                steps.append(lambda qq=qq: wo_out(qq))
                        return steps

                    def vproj_late(t):
                        vproj(t, False, wpp, tag="wp")

                    units = [("01", 0), ("01", 1), ("2", 0), ("01", 2),
                             ("2", 1), ("01", 3), ("2", 2), ("2", 3)]

                    # lag-1 pipeline, interleaved at step granularity: PE runs
                    # the previous unit's PV/Wo steps in the gaps between this
                    # unit's score groups (which are paced by ACT's exp).
                    prev_p = []
                    for i, u in enumerate(units):
                        last = i == len(units) - 1
                        S = s_steps(u, expts[i % 2])
                        if last:
                            # fold the final unit's own PV steps in behind its
                            # score groups (lag 2) so they don't pile up after
                            # the last exp: own chunk-step k needs exp group k.
                            own = p_steps(u, expts[i % 2], last=True)
                        done = 0
                        own_done = 0
                        for gi, s in enumerate(S):
                            s()
                            want = ((gi + 1) * len(prev_p)) // len(S)
                            while done < want:
                                prev_p[done]()
                                done += 1
                            if last and gi >= 2:
                                # own[0] is setup; chunk-step k is own[1+k]
                                while own_done < min(gi - 1, len(S) - 1) + 1:
                                    own[own_done]()
                                    own_done += 1
                        while done < len(prev_p):
                            prev_p[done]()
                            done += 1
                        if last:
                            prev_p = own[own_done:]
                        else:
                            prev_p = p_steps(u, expts[i % 2])
                            if i < 3:
                                prev_p = [lambda t=t: vproj_late(t)
                                          for t in range(4 + 4 * i, 8 + 4 * i)] + prev_p
                    for p in prev_p:
                        p()

    nc.compile()
    return nc


def _host_prep(x, Wqkv, Wo, seq_len):
    import ml_dtypes
    bf16 = ml_dtypes.bfloat16
    x = np.asarray(x, dtype=np.float32)
    Wqkv = np.asarray(Wqkv, dtype=np.float32)
    Wo = np.asarray(Wo, dtype=np.float32)
    off = int(np.asarray(seq_len).reshape(()))

    inv = 1.0 / (10000.0 ** (np.arange(0, DH, 2, dtype=np.float64) / DH))  # [32]
    pos = np.arange(T, dtype=np.float64) + off
    ang = pos[:, None] * inv[None, :]                 # [T, 32]
    cs = np.cos(ang).T                                # [32, T]
    sn = np.sin(ang).T
    cos128 = np.empty((128, T), np.float32)
    sin128 = np.empty((128, T), np.float32)
    for blk in range(2):
        r0 = blk * 64
        cos128[r0:r0 + 32] = cs
        cos128[r0 + 32:r0 + 64] = cs
        # sign-folded, NOT row-swapped (the swap happens on device): row r
        # multiplies the swapped operand X[sigma(r)], sign -1 on the first
        # half of each 64-row block.
        sin128[r0:r0 + 32] = -sn
        sin128[r0 + 32:r0 + 64] = sn

    in_maps = []
    for core in range(NC_):
        b, g = core // 4, core % 4
        hs = [3 * g, 3 * g + 1, 3 * g + 2]
        q = [Wqkv[:, h * DH:(h + 1) * DH] for h in hs]
        k = [Wqkv[:, C + h * DH:C + (h + 1) * DH] for h in hs]
        v = [Wqkv[:, 2 * C + h * DH:2 * C + (h + 1) * DH] for h in hs]
        wqkv_l = np.concatenate(
            [q[0], q[1], k[0], k[1], q[2], k[2], v[0], v[1], v[2]], axis=1)
        in_maps.append({
            "xT": np.ascontiguousarray(x[b].T).astype(bf16),
            "wqkv": np.ascontiguousarray(wqkv_l).astype(bf16),
            "wo": np.ascontiguousarray(
                Wo[g * HPC * DH:(g + 1) * HPC * DH, :]).astype(bf16),
            "cosT": cos128.astype(bf16),
            "sinT": sin128.astype(bf16),
        })
    return in_maps


def _run(in_maps, trace=False):
    global _prog
    from concourse.bass_utils import run_bass_kernel_spmd
    if _prog is None:
        _prog = _build()
    return run_bass_kernel_spmd(_prog, in_maps, list(range(NC_)), trace=trace)


def kernel(x, Wqkv, Wo, seq_len):
    in_maps = _host_prep(x, Wqkv, Wo, seq_len)
    res = _run(in_maps, trace=False)
    out = np.zeros((B, T, C), dtype=np.float32)
    for core in range(NC_):
        out[core // 4] += res.results[core]["out"].astype(np.float32)
    return out


# revision 13
# speedup vs baseline: 1.4838x; 1.0270x over previous
"""Distributed causal attention (qkv proj + RoPE + SDPA + out proj) on 8 trn2 cores.

Sharding: data-parallel over batch (B=2), tensor-parallel over heads
(12 heads -> 4 groups of 3). Core c handles batch c//4, heads 3*(c%4)..3*(c%4)+2.
Each core computes a partial output x_b @ Wqkv_heads -> attention -> @ Wo_rows;
the host sums the 4 head-group partials per batch (bf16 partials, fp32 sum).

Device layout per core (bf16 matmul operands, fp32 PSUM accumulation):
  xT    [768, 2048]  x[b] transposed (C-major), bf16
  wqkv  [768, 576]   columns: [q0 q1 | k0 k1 | q2 k2 | v0 v1 v2] (64 each), bf16
  wo    [192, 768]   Wo rows for the 3 heads, bf16
  cosT/sinT [128, 2048] RoPE tables, bf16 (sinT sign-folded; the rotate-half
  row swap happens on device via SBUF->SBUF DMA)
  out   [2048, 768]  bf16 partial (pre-reduction) output

Attention is a lag-1 software pipeline over units
  [(01,0), (01,1), (2,0), (01,2), (2,1), (01,3), (2,2), (2,3)]:
PE runs the scores matmuls of unit i+1 while ACT exponentiates unit i, then PE's
PV matmuls of unit i follow.  Unit (01,j) computes heads 0 and 1 together with
row-packed K=64 matmuls; unit (2,j) does head 2 alone, alternating row halves
via duplicated q2/k2.  After both units of query-block j are normalized, that
block's output projection + DMA-out run.

vs the 149us baseline: causal fine-trim at 128-query granularity (scores/exp/
PV only touch q' >= 128u of diagonal chunk 4j+u; [128,128] triangle masks),
RoPE row-swap via SBUF->SBUF DMA, PE clock-gate warm-up, rope tables DMAed
per T-half right after the h0 inputs, exp-only ACT during attention, bf16
output partials.
"""
import numpy as np

B, T, C = 2, 2048, 768
H, DH = 12, 64
HPC = 3            # heads per core
NC_ = 8            # cores
QB = 512           # query block
KC = 128           # key chunk
HF = T // 2
NJ = T // QB       # 4 query blocks
NKC = T // KC      # 16 key chunks
SCALE = 1.0 / float(np.sqrt(DH))

_prog = None


def _build():
    import concourse.bass as bass
    import concourse.tile as tile
    from concourse import bacc, mybir

    f32 = mybir.dt.float32
    bf16 = mybir.dt.bfloat16
    Exp = mybir.ActivationFunctionType.Exp

    nc = bacc.Bacc("TRN2", target_bir_lowering=False, debug=False)

    xT_p = nc.declare_dram_parameter("xT", [C, T], bf16, isOutput=False)
    wqkv_p = nc.declare_dram_parameter("wqkv", [C, 576], bf16, isOutput=False)
    wo_p = nc.declare_dram_parameter("wo", [HPC * DH, C], bf16, isOutput=False)
    cos_p = nc.declare_dram_parameter("cosT", [128, T], bf16, isOutput=False)
    sin_p = nc.declare_dram_parameter("sinT", [128, T], bf16, isOutput=False)
    out_p = nc.declare_dram_parameter("out", [T, C], bf16, isOutput=True)
    # DRAM bounce for the softmax-reciprocal partition-broadcast (SBUF APs
    # cannot have a zero partition step; DRAM APs can)
    recd_d = nc.dram_tensor("recd_dram", [1, HPC * QB], f32)

    with tile.TileContext(nc) as tc:
        with tc.tile_pool(name="persist", bufs=1) as persist:
            q01 = persist.tile([128, T], bf16, tag="q01")
            k01 = persist.tile([128, T], bf16, tag="k01")
            qk2 = persist.tile([128, T], bf16, tag="qk2")   # rows 0:64 q2, 64:128 q2 dup
            k2al = persist.tile([128, T], bf16, tag="k2al")  # rows 0:64 k2, 64:128 k2 dup
            vones = persist.tile([128, NKC, HPC, DH + 1], bf16, tag="vones")
            tri = persist.tile([128, KC], bf16, tag="tri")
            tri2 = persist.tile([128, 2 * KC], bf16, tag="tri2")
            warm = persist.tile([1, 16], f32, tag="warm")
            ones64f = persist.tile([1, 64], f32, tag="ones64f")
            wpe = persist.tile([1, 16], bf16, tag="wpe")
            wq = persist.tile([128, 6, 576], bf16, tag="wq")
            xts = [persist.tile([128, T], bf16, tag=f"xt{k}", name=f"xt{k}")
                   for k in range(6)]
            cosT = persist.tile([128, T], bf16, tag="cosT")
            sinT = persist.tile([128, T], bf16, tag="sinT")

            # preload the exp table set while DMAs run
            nc.vector.memset(warm, 0.0)
            nc.vector.memset(ones64f, 1.0)
            nc.vector.memset(wpe, 0.0)
            nc.scalar.activation(out=warm[0:1, 0:8], in_=warm[0:1, 0:8],
                                 func=Exp, scale=1.0)
            # causal triangle masks: tri[k, q'] = 1 if q' >= k else 0
            nc.gpsimd.memset(tri, 1.0)
            nc.gpsimd.affine_select(
                out=tri, in_=tri,
                compare_op=mybir.AluOpType.is_ge, fill=0.0, base=0,
                pattern=[[1, KC]], channel_multiplier=-1,
            )
            # tri2 = [zeros | tri] for the head-2 diagonal pair mask
            nc.gpsimd.memset(tri2[:, 0:KC], 0.0)
            nc.gpsimd.memset(tri2[:, KC:2 * KC], 1.0)
            nc.gpsimd.affine_select(
                out=tri2[:, KC:2 * KC], in_=tri2[:, KC:2 * KC],
                compare_op=mybir.AluOpType.is_ge, fill=0.0, base=0,
                pattern=[[1, KC]], channel_multiplier=-1,
            )
            # ones column of vones (for the fused softmax denominator)
            nc.gpsimd.memset(vones[:, :, :, DH:DH + 1], 1.0)

            h0 = slice(0, HF)
            h1 = slice(HF, T)

            rp_cm = tc.tile_pool(name="rp", bufs=2)
            rp = rp_cm.__enter__()
            with tc.tile_pool(name="pp", bufs=1, space="PSUM") as pp, \
                 tc.tile_pool(name="vp", bufs=2, space="PSUM") as vp, \
                 tc.tile_pool(name="wmp", bufs=1, space="PSUM") as wmp:
                # PE warm-up: tiny matmuls fill the DMA wait and release the
                # HAM clock throttle before the first projection matmul
                wps = wmp.tile([1, 16], f32, tag="wps")
                for _ in range(80):
                    nc.tensor.matmul(wps, lhsT=wpe[0:1, 0:1], rhs=wpe[0:1, :],
                                     start=True, stop=True)

                # input DMAs, in first-use order: h0 of everything, then h1
                for k in range(6):
                    nc.gpsimd.dma_start(
                        out=wq[:, k, :], in_=wqkv_p[k * 128:(k + 1) * 128, :])
                    nc.sync.dma_start(out=xts[k][:, h0],
                                      in_=xT_p[k * 128:(k + 1) * 128, h0])
                nc.gpsimd.dma_start(out=cosT[:, h0], in_=cos_p[:, h0])
                nc.gpsimd.dma_start(out=sinT[:, h0], in_=sin_p[:, h0])
                for k in range(6):
                    nc.sync.dma_start(out=xts[k][:, h1],
                                      in_=xT_p[k * 128:(k + 1) * 128, h1])
                nc.gpsimd.dma_start(out=cosT[:, h1], in_=cos_p[:, h1])
                nc.gpsimd.dma_start(out=sinT[:, h1], in_=sin_p[:, h1])

                def rope(X, out_q, out_k, sl):
                    """RoPE X[:, sl] in place (or, for the q2k2 tile, into
                    out_q/out_k rows 0:64).  Rotate-half row swap via 4
                    SBUF->SBUF DMAs; sin sign-folding is in the host table."""
                    w = sl.stop - sl.start
                    tp = rp.tile([128, HF], bf16, tag="tp")
                    for r in (0, 64):
                        nc.gpsimd.dma_start(out=tp[r:r + 32, 0:w],
                                            in_=X[r + 32:r + 64, sl])
                        nc.gpsimd.dma_start(out=tp[r + 32:r + 64, 0:w],
                                            in_=X[r:r + 32, sl])
                    nc.vector.tensor_mul(tp[:, 0:w], tp[:, 0:w], sinT[:, sl])
                    if out_k is None:
                        nc.vector.tensor_mul(X[:, sl], X[:, sl], cosT[:, sl])
                        nc.vector.tensor_add(X[:, sl], X[:, sl], tp[:, 0:w])
                    else:
                        # cos product into a full-height scratch so the adds
                        # have base-partition-aligned inputs
                        ct = rp.tile([128, HF], bf16, tag="tp")
                        nc.vector.tensor_mul(ct[:, 0:w], X[:, sl],
                                             cosT[:, sl])
                        nc.vector.tensor_add(out_q[0:64, sl], ct[0:64, 0:w],
                                             tp[0:64, 0:w])
                        nc.vector.tensor_add(out_k[0:64, sl], ct[64:128, 0:w],
                                             tp[64:128, 0:w])

                # q/k projection: M-tile m of qkvT = wqkv cols [128m, 128m+128)
                def proj_qk(m, X, out_q=None, out_k=None, half=0):
                    pst = []
                    for nn in range(2):
                        ps = pp.tile([128, QB], f32, tag=f"pp{nn}",
                                     name=f"pp{m}_{half}_{nn}")
                        pst.append(ps)
                    for k in range(6):
                        for nn in range(2):
                            n = half * 2 + nn
                            nc.tensor.matmul(
                                pst[nn],
                                lhsT=wq[:, k, m * 128:(m + 1) * 128],
                                rhs=xts[k][:, n * QB:(n + 1) * QB],
                                start=(k == 0), stop=(k == 5))
                    for nn in range(2):
                        n = half * 2 + nn
                        nc.scalar.copy(X[:, n * QB:(n + 1) * QB], pst[nn])
                    rope(X, out_q, out_k,
                         slice(half * HF, (half + 1) * HF))

                def vproj(t, on_act, pool, tag="vp"):
                    ps = pool.tile([128, 192], f32, tag=tag, name=f"vps{t}")
                    for k in range(6):
                        nc.tensor.matmul(
                            ps, lhsT=xts[k][:, t * 128:(t + 1) * 128],
                            rhs=wq[:, k, 384:576],
                            start=(k == 0), stop=(k == 5))
                    cp = nc.scalar.copy if on_act else nc.vector.tensor_copy
                    cp(vones[:, t, :, 0:DH],
                       ps.rearrange("p (h d) -> p h d", h=HPC))

                # h0 of q01+k01 first (unblocks the first score units), then
                # the v chunks they need, then everything else
                proj_qk(0, q01, half=0)
                proj_qk(1, k01, half=0)
                for t in range(4):
                    vproj(t, True, vp)
                proj_qk(2, qk2, out_q=qk2, out_k=k2al, half=0)
                nc.gpsimd.dma_start(out=qk2[64:128, h0], in_=qk2[0:64, h0])
                nc.gpsimd.dma_start(out=k2al[64:128, h0], in_=k2al[0:64, h0])

            # --- attention + per-block output projection ---
            with tc.tile_pool(name="phaseB", bufs=1) as pb, \
                 tc.tile_pool(name="bct", bufs=2) as bcp, \
                 tc.tile_pool(name="ostage", bufs=3) as osp:
                expts = [pb.tile([128, 2, NKC, QB], bf16, name=f"expt{i}",
                                 tag=f"expt{i}")
                         for i in range(2)]
                outt01 = pb.tile([128, T], bf16, tag="outt01")
                outt2 = pb.tile([64, T], bf16, tag="outt2")
                # denominators, row r = j*HPC + h; recd = 1/denom
                denom = pb.tile([1, HPC * QB], f32, tag="denom")
                recd = pb.tile([1, HPC * QB], f32, tag="recd")
                wo01 = pb.tile([128, C], bf16, tag="wo01")
                nc.gpsimd.dma_start(out=wo01, in_=wo_p[0:128, :])
                wo2 = pb.tile([64, C], bf16, tag="wo2")
                nc.gpsimd.dma_start(out=wo2, in_=wo_p[128:192, :])

                def tgt_of(h):
                    return outt01[0:64] if h == 0 else (outt01[64:128] if h == 1 else outt2[0:64])

                with tc.tile_pool(name="sc", bufs=2, space="PSUM") as scp, \
                     tc.tile_pool(name="pv", bufs=2, space="PSUM") as pvp, \
                     tc.tile_pool(name="wp", bufs=1, space="PSUM") as wpp:

                    def s_steps(unit, expt):
                        """Closures: one per 2-matmul scores psum group (+exp,
                        +triangle masks on diagonal chunks).  Diagonal chunk
                        c=4j+u only computes queries q' >= 128u."""
                        hh, j = unit
                        steps = []
                        if hh == "01":
                            # heads 0+1 row-packed: per sc tile, 1 chunk each
                            def grp01(c):
                                u = c - 4 * j
                                off = KC * u if u > 0 else 0
                                qsl = slice(j * QB + off, (j + 1) * QB)
                                sc = scp.tile([128, 2, QB], f32, tag="sc",
                                              name=f"sc01_{j}_{c}")
                                nc.tensor.matmul(
                                    sc[:, 0, off:QB],
                                    lhsT=k01[0:64, c * KC:(c + 1) * KC],
                                    rhs=q01[0:64, qsl],
                                    start=True, stop=True)
                                nc.tensor.matmul(
                                    sc[:, 1, off:QB],
                                    lhsT=k01[64:128, c * KC:(c + 1) * KC],
                                    rhs=q01[64:128, qsl],
                                    start=True, stop=True)
                                nc.scalar.activation(
                                    out=expt[:, :, c, off:QB],
                                    in_=sc[:, :, off:QB],
                                    func=Exp, scale=SCALE)
                                if u >= 0:
                                    for hh_ in range(2):
                                        nc.vector.tensor_mul(
                                            expt[:, hh_, c, off:off + KC],
                                            expt[:, hh_, c, off:off + KC],
                                            tri)
                            for c in range(4 * (j + 1)):
                                steps.append(lambda c=c: grp01(c))
                        else:
                            # head 2: alternate row halves for LDW/MM overlap
                            def grp2(g):
                                c0 = 2 * g
                                u0 = c0 - 4 * j
                                off = KC * u0 if u0 > 0 else 0
                                qsl = slice(j * QB + off, (j + 1) * QB)
                                sc = scp.tile([128, 2, QB], f32, tag="sc",
                                              name=f"sc2_{j}_{g}")
                                for uu in range(2):
                                    c = c0 + uu
                                    lo = c % 2 == 0
                                    kk = k2al[0:64] if lo else k2al[64:128]
                                    qq = qk2[0:64] if lo else qk2[64:128]
                                    nc.tensor.matmul(
                                        sc[:, uu, off:QB],
                                        lhsT=kk[:, c * KC:(c + 1) * KC],
                                        rhs=qq[:, qsl],
                                        start=True, stop=True)
                                nc.scalar.activation(
                                    out=expt[:, 0, c0:c0 + 2, off:QB],
                                    in_=sc[:, :, off:QB],
                                    func=Exp, scale=SCALE)
                                if u0 >= 0:
                                    # c0: triangle; c0+1: zero+triangle
                                    nc.vector.tensor_mul(
                                        expt[:, 0, c0, off:off + KC],
                                        expt[:, 0, c0, off:off + KC],
                                        tri)
                                    nc.vector.tensor_mul(
                                        expt[:, 0, c0 + 1, off:off + 2 * KC],
                                        expt[:, 0, c0 + 1, off:off + 2 * KC],
                                        tri2)
                            for g in range(2 * (j + 1)):
                                steps.append(lambda g=g: grp2(g))
                        return steps

                    def p_steps(unit, expt, last=False):
                        """Closures: PV matmul chunk-steps, then copy+normalize,
                        then (after the '2' unit) the block's output projection."""
                        hh, j = unit
                        nch = 4 * (j + 1)
                        heads = [(0, 0), (1, 1)] if hh == "01" else [(2, 0)]
                        pos = {}
                        steps = []

                        def setup():
                            for h, _ in heads:
                                pos[h] = pvp.tile([128, QB], f32, tag="pv",
                                                  name=f"po_{h}_{j}")

                        def chunk(c):
                            u = c - 4 * j
                            off = KC * u if u > 0 else 0
                            for h, hh_slot in heads:
                                nc.tensor.matmul(
                                    pos[h][0:DH + 1, off:QB],
                                    lhsT=vones[:, c, h, :],
                                    rhs=expt[:, hh_slot, c, off:QB],
                                    start=(c == 0), stop=(c == nch - 1),
                                    skip_group_check=True)

                        steps.append(setup)
                        for c0 in range(0, nch, 2):
                            def two(c0=c0):
                                chunk(c0)
                                chunk(c0 + 1)
                            steps.append(two)

                        def fin(h, hh_slot):
                            po = pos[h]
                            nc.vector.tensor_copy(
                                tgt_of(h)[:, j * QB:(j + 1) * QB], po[0:DH, :])
                            nc.vector.tensor_copy(
                                denom[0:1, h * QB:(h + 1) * QB],
                                po[DH:DH + 1, :])

                        def norm_unit():
                            usl = slice(heads[0][0] * QB,
                                        (heads[-1][0] + 1) * QB)
                            with nc.allow_low_precision(reason="softmax denom reciprocal: 18-bit approx"):
                                nc.vector.reciprocal_approx_fast(
                                    out=recd[0:1, usl], in_=denom[0:1, usl])
                            if last:
                                # scores are done: broadcast 1/denom across
                                # partitions with a K=1 matmul into a free sc
                                # psum slot instead of the DRAM round-trip
                                for h, _ in heads:
                                    bctp = scp.tile([128, QB], f32, tag="sc",
                                                    name=f"bctp_{h}_{j}")
                                    nc.tensor.matmul(
                                        bctp[0:64, :], lhsT=ones64f[0:1, :],
                                        rhs=recd[0:1, h * QB:(h + 1) * QB],
                                        start=True, stop=True)
                                    tgt = tgt_of(h)
                                    sl = slice(j * QB, (j + 1) * QB)
                                    nc.vector.tensor_mul(
                                        tgt[:, sl], tgt[:, sl], bctp[0:64, :])
                                return
                            nc.sync.dma_start(out=recd_d[0:1, usl],
                                              in_=recd[0:1, usl])
                            for h, _ in heads:
                                base = 64 if h == 1 else 0
                                src = recd_d[0:1, h * QB:(h + 1) * QB]
                                bsrc = bass.AP(
                                    tensor=src.tensor, offset=src.offset,
                                    ap=[[0, 64]] + list(src.ap[1:]))
                                bct = bcp.tile([128, QB], f32, tag="bct",
                                               name=f"bct_{h}_{j}")
                                nc.sync.dma_start(
                                    out=bct[base:base + 64, :], in_=bsrc)
                                tgt = tgt_of(h)
                                sl = slice(j * QB, (j + 1) * QB)
                                nc.vector.tensor_mul(
                                    tgt[:, sl], tgt[:, sl], bct[base:base + 64, :])

                        for h, hh_slot in heads:
                            steps.append(lambda h=h, s=hh_slot: fin(h, s))
                        steps.append(norm_unit)

                        if hh == "2":
                            pws = {}

                            def wo_mm(qq):
                                q = j * 4 + qq
                                if last and qq % 2 == 1:
                                    pwA = pvp.tile([128, QB], f32, tag="pv",
                                                   name=f"pwA_{q}")
                                    pwB = pvp.tile([128, 256], f32, tag="pv",
                                                   name=f"pwB_{q}")
                                else:
                                    pw = wpp.tile([128, 1024], f32, tag="wp",
                                                  name=f"pw_{q}")
                                    pwA, pwB = pw[:, 0:QB], pw[:, QB:C]
                                pws[qq] = (pwA, pwB)
                                for dst, (n0, n1) in ((pwA, (0, 512)),
                                                      (pwB, (512, 768))):
                                    nc.tensor.matmul(
                                        dst,
                                        lhsT=outt01[:, q * 128:(q + 1) * 128],
                                        rhs=wo01[:, n0:n1],
                                        start=True, stop=False)
                                    nc.tensor.matmul(
                                        dst,
                                        lhsT=outt2[:, q * 128:(q + 1) * 128],
                                        rhs=wo2[:, n0:n1],
                                        start=False, stop=True)

                            def wo_out(qq):
                                q = j * 4 + qq
                                pwA, pwB = pws[qq]
                                ot = osp.tile([128, C], bf16, tag="ot",
                                              name=f"ot_{q}")
                                cpf = (nc.scalar.copy if last
                                       else nc.vector.tensor_copy)
                                cpf(ot[:, 0:QB], pwA)
                                cpf(ot[:, QB:C], pwB)
                                nc.sync.dma_start(
                                    out=out_p[q * 128:(q + 1) * 128, :], in_=ot)
                            for qq in range(4):
                                steps.append(lambda qq=qq: wo_mm(qq))
                                steps.append(lambda qq=qq: wo_out(qq))
                        return steps

                    def vproj_late(t):
                        vproj(t, False, wpp, tag="wp")

                    def proj_h1(m):
                        """h1-half q/k projection, run as attention filler:
                        psum from the wo pool, copies on DVE."""
                        X = q01 if m == 0 else (k01 if m == 1 else qk2)
                        ps = wpp.tile([128, 1024], f32, tag="wp",
                                      name=f"pph1_{m}")
                        for k in range(6):
                            for nn in range(2):
                                n = 2 + nn
                                nc.tensor.matmul(
                                    ps[:, nn * QB:(nn + 1) * QB],
                                    lhsT=wq[:, k, m * 128:(m + 1) * 128],
                                    rhs=xts[k][:, n * QB:(n + 1) * QB],
                                    start=(k == 0), stop=(k == 5))
                        for nn in range(2):
                            n = 2 + nn
                            nc.vector.tensor_copy(
                                X[:, n * QB:(n + 1) * QB],
                                ps[:, nn * QB:(nn + 1) * QB])
                        if m == 2:
                            rope(qk2, qk2, k2al, h1)
                            nc.gpsimd.dma_start(out=qk2[64:128, h1],
                                                in_=qk2[0:64, h1])
                            nc.gpsimd.dma_start(out=k2al[64:128, h1],
                                                in_=k2al[0:64, h1])
                        else:
                            rope(X, None, None, h1)

                    units = [("01", 0), ("01", 1), ("2", 0), ("2", 1),
                             ("01", 2), ("2", 2), ("01", 3), ("2", 3)]

                    # lag-1 pipeline, interleaved at step granularity: PE runs
                    # the previous unit's PV/Wo steps in the gaps between this
                    # unit's score groups (which are paced by ACT's exp).
                    prev_p = []
                    for i, u in enumerate(units):
                        last = i == len(units) - 1
                        S = s_steps(u, expts[i % 2])
                        if last:
                            # fold the final unit's own PV steps in behind its
                            # score groups (lag 2) so they don't pile up after
                            # the last exp: own chunk-step k needs exp group k.
                            own = p_steps(u, expts[i % 2], last=True)
                        done = 0
                        own_done = 0
                        for gi, s in enumerate(S):
                            s()
                            want = ((gi + 1) * len(prev_p)) // len(S)
                            while done < want:
                                prev_p[done]()
                                done += 1
                            if last and gi >= 2:
                                # own[0] is setup; chunk-step k is own[1+k]
                                while own_done < min(gi - 1, len(S) - 1) + 1:
                                    own[own_done]()
                                    own_done += 1
                        while done < len(prev_p):
                            prev_p[done]()
                            done += 1
                        if last:
                            prev_p = own[own_done:]
                        else:
                            prev_p = p_steps(u, expts[i % 2])
                            if i < 3:
                                prev_p = ([lambda t=t: vproj_late(t)
                                           for t in range(4 + 4 * i, 8 + 4 * i)]
                                          + prev_p
                                          + [lambda m=i: proj_h1(m)])
                    for p in prev_p:
                        p()

            rp_cm.__exit__(None, None, None)

    nc.compile()
    return nc


def _host_prep(x, Wqkv, Wo, seq_len):
    import ml_dtypes
    bf16 = ml_dtypes.bfloat16
    x = np.asarray(x, dtype=np.float32)
    Wqkv = np.asarray(Wqkv, dtype=np.float32)
    Wo = np.asarray(Wo, dtype=np.float32)
    off = int(np.asarray(seq_len).reshape(()))

    inv = 1.0 / (10000.0 ** (np.arange(0, DH, 2, dtype=np.float64) / DH))  # [32]
    pos = np.arange(T, dtype=np.float64) + off
    ang = pos[:, None] * inv[None, :]                 # [T, 32]
    cs = np.cos(ang).T                                # [32, T]
    sn = np.sin(ang).T
    cos128 = np.empty((128, T), np.float32)
    sin128 = np.empty((128, T), np.float32)
    for blk in range(2):
        r0 = blk * 64
        cos128[r0:r0 + 32] = cs
        cos128[r0 + 32:r0 + 64] = cs
        # sign-folded, NOT row-swapped (the swap happens on device): row r
        # multiplies the swapped operand X[sigma(r)], sign -1 on the first
        # half of each 64-row block.
        sin128[r0:r0 + 32] = -sn
        sin128[r0 + 32:r0 + 64] = sn

    in_maps = []
    for core in range(NC_):
        b, g = core // 4, core % 4
        hs = [3 * g, 3 * g + 1, 3 * g + 2]
        q = [Wqkv[:, h * DH:(h + 1) * DH] for h in hs]
        k = [Wqkv[:, C + h * DH:C + (h + 1) * DH] for h in hs]
        v = [Wqkv[:, 2 * C + h * DH:2 * C + (h + 1) * DH] for h in hs]
        wqkv_l = np.concatenate(
            [q[0], q[1], k[0], k[1], q[2], k[2], v[0], v[1], v[2]], axis=1)
        in_maps.append({
            "xT": np.ascontiguousarray(x[b].T).astype(bf16),
            "wqkv": np.ascontiguousarray(wqkv_l).astype(bf16),
            "wo": np.ascontiguousarray(
                Wo[g * HPC * DH:(g + 1) * HPC * DH, :]).astype(bf16),
            "cosT": cos128.astype(bf16),
            "sinT": sin128.astype(bf16),
        })
    return in_maps


def _run(in_maps, trace=False):
    global _prog
    from concourse.bass_utils import run_bass_kernel_spmd
    if _prog is None:
        _prog = _build()
    return run_bass_kernel_spmd(_prog, in_maps, list(range(NC_)), trace=trace)


def kernel(x, Wqkv, Wo, seq_len):
    in_maps = _host_prep(x, Wqkv, Wo, seq_len)
    res = _run(in_maps, trace=False)
    out = np.zeros((B, T, C), dtype=np.float32)
    for core in range(NC_):
        out[core // 4] += res.results[core]["out"].astype(np.float32)
    return out


# revision 16
# speedup vs baseline: 1.4897x; 1.0040x over previous
"""Distributed causal attention (qkv proj + RoPE + SDPA + out proj) on 8 trn2 cores.

Sharding: data-parallel over batch (B=2), tensor-parallel over heads
(12 heads -> 4 groups of 3). Core c handles batch c//4, heads 3*(c%4)..3*(c%4)+2.
Each core computes a partial output x_b @ Wqkv_heads -> attention -> @ Wo_rows;
the host sums the 4 head-group partials per batch (bf16 partials, fp32 sum).

Key structure (evolved from the 149us baseline):
- q/k tensors live in PER-HALF tiles (q01h[0] = queries 0:1024, q01h[1] =
  1024:2048, same for k01h/qk2h/k2alh).  The h1-half projections run as
  attention-phase fillers; separate tiles mean the h0 readers never falsely
  serialize against the h1 writers (the tile framework tracks dependencies
  at tile granularity).
- Unit order [(01,0),(01,1),(2,0),(2,1),(01,2),(2,2),(01,3),(2,3)]: the four
  h0-only units run first while the h1 halves are produced underneath them.
- Causal fine-trim at 128-query granularity: for diagonal chunk c = 4j+u
  only queries q' >= 128u are computed (scores matmul N, exp N, PV matmul N);
  the remaining triangle is masked with a [128,128] multiply.
- ACT runs exp only during the attention phase; Wo/v-proj copies are DVE;
  the final unit's Wo copies are ACT (idle after the last exp).
- Wo output projections are double-tracked through the wo psum pool (even
  q-tiles) and the pv pool (odd q-tiles) so the psum-cast WAR chains overlap;
  wo(j) work is explicitly placed under units with exp slack.
- The last unit's softmax normalization broadcasts 1/denom with a K=1 matmul
  into a free scores-psum slot instead of the DRAM bounce.
- PE warm-up matmuls at t=0 release the HAM clock gate during the DMA wait.
- bf16 output partials (halves the out DMA).
"""
import numpy as np

B, T, C = 2, 2048, 768
H, DH = 12, 64
HPC = 3            # heads per core
NC_ = 8            # cores
QB = 512           # query block
KC = 128           # key chunk
HF = T // 2
NJ = T // QB       # 4 query blocks
NKC = T // KC      # 16 key chunks
SCALE = 1.0 / float(np.sqrt(DH))

_prog = None


def _build():
    import concourse.bass as bass
    import concourse.tile as tile
    from concourse import bacc, mybir

    f32 = mybir.dt.float32
    bf16 = mybir.dt.bfloat16
    Exp = mybir.ActivationFunctionType.Exp

    nc = bacc.Bacc("TRN2", target_bir_lowering=False, debug=False)

    xT_p = nc.declare_dram_parameter("xT", [C, T], bf16, isOutput=False)
    wqkv_p = nc.declare_dram_parameter("wqkv", [C, 576], bf16, isOutput=False)
    wo_p = nc.declare_dram_parameter("wo", [HPC * DH, C], bf16, isOutput=False)
    cos_p = nc.declare_dram_parameter("cosT", [128, T], bf16, isOutput=False)
    sin_p = nc.declare_dram_parameter("sinT", [128, T], bf16, isOutput=False)
    out_p = nc.declare_dram_parameter("out", [T, C], bf16, isOutput=True)
    # DRAM bounce for the softmax-reciprocal partition-broadcast (SBUF APs
    # cannot have a zero partition step; DRAM APs can)
    recd_d = nc.dram_tensor("recd_dram", [1, HPC * QB], f32)

    with tile.TileContext(nc) as tc:
        with tc.tile_pool(name="persist", bufs=1) as persist:
            q01h = [persist.tile([128, HF], bf16, tag=f"q01_{i}", name=f"q01_{i}") for i in (0, 1)]
            k01h = [persist.tile([128, HF], bf16, tag=f"k01_{i}", name=f"k01_{i}") for i in (0, 1)]
            qk2h = [persist.tile([128, HF], bf16, tag=f"qk2_{i}", name=f"qk2_{i}") for i in (0, 1)]
            k2alh = [persist.tile([128, HF], bf16, tag=f"k2al_{i}", name=f"k2al_{i}") for i in (0, 1)]
            vones = persist.tile([128, NKC, HPC, DH + 1], bf16, tag="vones")
            tri = persist.tile([128, KC], bf16, tag="tri")
            tri2 = persist.tile([128, 2 * KC], bf16, tag="tri2")
            warm = persist.tile([1, 16], f32, tag="warm")
            ones64f = persist.tile([1, 64], f32, tag="ones64f")
            wpe = persist.tile([1, 16], bf16, tag="wpe")
            wq = persist.tile([128, 6, 576], bf16, tag="wq")
            xts = [persist.tile([128, T], bf16, tag=f"xt{k}", name=f"xt{k}")
                   for k in range(6)]
            cosT = persist.tile([128, T], bf16, tag="cosT")
            sinT = persist.tile([128, T], bf16, tag="sinT")

            # preload the exp table set while DMAs run
            nc.vector.memset(warm, 0.0)
            nc.vector.memset(wpe, 0.0)
            nc.vector.memset(ones64f, 1.0)
            nc.scalar.activation(out=warm[0:1, 0:8], in_=warm[0:1, 0:8],
                                 func=Exp, scale=1.0)
            # causal triangle masks: tri[k, q'] = 1 if q' >= k else 0
            nc.gpsimd.memset(tri, 1.0)
            nc.gpsimd.affine_select(
                out=tri, in_=tri,
                compare_op=mybir.AluOpType.is_ge, fill=0.0, base=0,
                pattern=[[1, KC]], channel_multiplier=-1,
            )
            # tri2 = [zeros | tri] for the head-2 diagonal pair mask
            nc.gpsimd.memset(tri2[:, 0:KC], 0.0)
            nc.gpsimd.memset(tri2[:, KC:2 * KC], 1.0)
            nc.gpsimd.affine_select(
                out=tri2[:, KC:2 * KC], in_=tri2[:, KC:2 * KC],
                compare_op=mybir.AluOpType.is_ge, fill=0.0, base=0,
                pattern=[[1, KC]], channel_multiplier=-1,
            )
            # ones column of vones (for the fused softmax denominator)
            nc.gpsimd.memset(vones[:, :, :, DH:DH + 1], 1.0)

            h0 = slice(0, HF)
            h1 = slice(HF, T)

            rp_cm = tc.tile_pool(name="rp", bufs=2)
            rp = rp_cm.__enter__()

            def rope(X, out_q, out_k, half):
                """RoPE the [128, HF] half-tile X in place (or, for the q2k2
                tile, rows 0:64 into out_q/out_k).  sinT is row-swapped +
                sign-folded so each multiply reads in0/in1 at the same base
                partition."""
                g = slice(half * HF, (half + 1) * HF)
                tp = rp.tile([128, HF], bf16, tag="tp")
                nc.vector.tensor_mul(tp[0:32], X[32:64, :], sinT[32:64, g])
                nc.vector.tensor_mul(tp[32:64], X[0:32, :], sinT[0:32, g])
                nc.vector.tensor_mul(tp[64:96], X[96:128, :], sinT[96:128, g])
                nc.vector.tensor_mul(tp[96:128], X[64:96, :], sinT[64:96, g])
                nc.vector.tensor_mul(X[:, :], X[:, :], cosT[:, g])
                if out_k is None:
                    nc.vector.tensor_add(X[:, :], X[:, :], tp)
                else:
                    nc.vector.tensor_add(out_q[0:64, :], X[0:64, :], tp[0:64])
                    nc.vector.tensor_add(out_k[0:64, :], X[64:128, :],
                                         tp[64:128])

            def emit_proj(m, half, pst, on_act):
                """qkvT M-tile m for T-half `half`: 12 N=512 matmuls into the
                two psum tiles pst, copy out, RoPE (+ head-2 row dup)."""
                for k in range(6):
                    for nn in range(2):
                        nc.tensor.matmul(
                            pst[nn],
                            lhsT=wq[:, k, m * 128:(m + 1) * 128],
                            rhs=xts[k][:, half * HF + nn * QB:
                                       half * HF + (nn + 1) * QB],
                            start=(k == 0), stop=(k == 5))
                cp = nc.scalar.copy if on_act else nc.vector.tensor_copy
                X = (q01h if m == 0 else (k01h if m == 1 else qk2h))[half]
                for nn in range(2):
                    cp(X[:, nn * QB:(nn + 1) * QB], pst[nn])
                if m == 2:
                    rope(X, X, k2alh[half], half)
                    nc.sync.dma_start(out=X[64:128, :], in_=X[0:64, :])
                    nc.sync.dma_start(out=k2alh[half][64:128, :],
                                      in_=k2alh[half][0:64, :])
                else:
                    rope(X, None, None, half)

            with tc.tile_pool(name="pp", bufs=1, space="PSUM") as pp, \
                 tc.tile_pool(name="vp", bufs=2, space="PSUM") as vp, \
                 tc.tile_pool(name="wmp", bufs=1, space="PSUM") as wmp:
                # PE warm-up: tiny matmuls fill the DMA wait and release the
                # HAM clock throttle before the first projection matmul
                wps = wmp.tile([1, 16], f32, tag="wps")
                for _ in range(80):
                    nc.tensor.matmul(wps, lhsT=wpe[0:1, 0:1], rhs=wpe[0:1, :],
                                     start=True, stop=True)

                # input DMAs, in first-use order: h0 of everything, then h1
                for k in range(6):
                    nc.sync.dma_start(
                        out=wq[:, k, :], in_=wqkv_p[k * 128:(k + 1) * 128, :])
                    nc.sync.dma_start(out=xts[k][:, h0],
                                      in_=xT_p[k * 128:(k + 1) * 128, h0])
                nc.sync.dma_start(out=cosT[:, h0], in_=cos_p[:, h0])
                nc.sync.dma_start(out=sinT[:, h0], in_=sin_p[:, h0])
                for k in range(6):
                    nc.sync.dma_start(out=xts[k][:, h1],
                                      in_=xT_p[k * 128:(k + 1) * 128, h1])
                nc.sync.dma_start(out=cosT[:, h1], in_=cos_p[:, h1])
                nc.sync.dma_start(out=sinT[:, h1], in_=sin_p[:, h1])

                def proj_qk(m, half):
                    pst = [pp.tile([128, QB], f32, tag=f"pp{nn}",
                                   name=f"pp{m}_{half}_{nn}")
                           for nn in range(2)]
                    emit_proj(m, half, pst, on_act=True)

                def vproj(t, on_act, pool, tag="vp"):
                    ps = pool.tile([128, 192], f32, tag=tag, name=f"vps{t}")
                    for k in range(6):
                        nc.tensor.matmul(
                            ps, lhsT=xts[k][:, t * 128:(t + 1) * 128],
                            rhs=wq[:, k, 384:576],
                            start=(k == 0), stop=(k == 5))
                    cp = nc.scalar.copy if on_act else nc.vector.tensor_copy
                    cp(vones[:, t, :, 0:DH],
                       ps.rearrange("p (h d) -> p h d", h=HPC))

                # critical path to the first units: h0 projections only
                proj_qk(0, 0)
                proj_qk(1, 0)
                for t in range(4):
                    vproj(t, True, vp)
                proj_qk(2, 0)

            # --- attention + per-block output projection ---
            with tc.tile_pool(name="phaseB", bufs=1) as pb, \
                 tc.tile_pool(name="bct", bufs=2) as bcp, \
                 tc.tile_pool(name="ostage", bufs=3) as osp:
                expts = [pb.tile([128, 2, NKC, QB], bf16, name=f"expt{i}",
                                 tag=f"expt{i}")
                         for i in range(2)]
                outt01 = pb.tile([128, T], bf16, tag="outt01")
                outt2 = pb.tile([64, T], bf16, tag="outt2")
                denom = pb.tile([1, HPC * QB], f32, tag="denom")
                recd = pb.tile([1, HPC * QB], f32, tag="recd")
                wo01 = pb.tile([128, C], bf16, tag="wo01")
                nc.sync.dma_start(out=wo01, in_=wo_p[0:128, :])
                wo2 = pb.tile([64, C], bf16, tag="wo2")
                nc.sync.dma_start(out=wo2, in_=wo_p[128:192, :])

                def tgt_of(h):
                    return outt01[0:64] if h == 0 else (outt01[64:128] if h == 1 else outt2[0:64])

                with tc.tile_pool(name="sc", bufs=2, space="PSUM") as scp, \
                     tc.tile_pool(name="pv", bufs=2, space="PSUM") as pvp, \
                     tc.tile_pool(name="wp", bufs=1, space="PSUM") as wpp:

                    def s_steps(unit, expt):
                        """Score-group closures: 2 matmuls + exp (+ causal
                        triangle masks), fine-trimmed on diagonal chunks."""
                        hh, j = unit
                        jh, lj = divmod(j, 2)
                        steps = []
                        if hh == "01":
                            def grp01(c):
                                u = c - 4 * j
                                off = KC * u if u > 0 else 0
                                ch, lc = divmod(c, 8)
                                ksl = slice(lc * KC, (lc + 1) * KC)
                                qsl = slice(lj * QB + off, (lj + 1) * QB)
                                sc = scp.tile([128, 2, QB], f32, tag="sc",
                                              name=f"sc01_{j}_{c}")
                                nc.tensor.matmul(
                                    sc[:, 0, off:QB],
                                    lhsT=k01h[ch][0:64, ksl],
                                    rhs=q01h[jh][0:64, qsl],
                                    start=True, stop=True)
                                nc.tensor.matmul(
                                    sc[:, 1, off:QB],
                                    lhsT=k01h[ch][64:128, ksl],
                                    rhs=q01h[jh][64:128, qsl],
                                    start=True, stop=True)
                                nc.scalar.activation(
                                    out=expt[:, :, c, off:QB],
                                    in_=sc[:, :, off:QB],
                                    func=Exp, scale=SCALE)
                                if u >= 0:
                                    for hh_ in range(2):
                                        nc.vector.tensor_mul(
                                            expt[:, hh_, c, off:off + KC],
                                            expt[:, hh_, c, off:off + KC],
                                            tri)
                            for c in range(4 * (j + 1)):
                                steps.append(lambda c=c: grp01(c))
                        else:
                            def grp2(g):
                                c0 = 2 * g
                                u0 = c0 - 4 * j
                                off = KC * u0 if u0 > 0 else 0
                                qsl = slice(lj * QB + off, (lj + 1) * QB)
                                sc = scp.tile([128, 2, QB], f32, tag="sc",
                                              name=f"sc2_{j}_{g}")
                                for uu in range(2):
                                    c = c0 + uu
                                    ch, lc = divmod(c, 8)
                                    ksl = slice(lc * KC, (lc + 1) * KC)
                                    lo = c % 2 == 0
                                    kk = (k2alh[ch][0:64] if lo
                                          else k2alh[ch][64:128])
                                    qq = (qk2h[jh][0:64] if lo
                                          else qk2h[jh][64:128])
                                    nc.tensor.matmul(
                                        sc[:, uu, off:QB],
                                        lhsT=kk[:, ksl], rhs=qq[:, qsl],
                                        start=True, stop=True)
                                nc.scalar.activation(
                                    out=expt[:, 0, c0:c0 + 2, off:QB],
                                    in_=sc[:, :, off:QB],
                                    func=Exp, scale=SCALE)
                                if u0 >= 0:
                                    nc.vector.tensor_mul(
                                        expt[:, 0, c0, off:off + KC],
                                        expt[:, 0, c0, off:off + KC],
                                        tri)
                                    nc.vector.tensor_mul(
                                        expt[:, 0, c0 + 1, off:off + 2 * KC],
                                        expt[:, 0, c0 + 1, off:off + 2 * KC],
                                        tri2)
                            for g in range(2 * (j + 1)):
                                steps.append(lambda g=g: grp2(g))
                        return steps

                    def wo_steps(j, last=False):
                        """Output projection for query block j: 4 q-tiles,
                        even ones through the wo psum pool, odd through the
                        pv pool, so the two psum-cast WAR chains overlap."""
                        pws = {}
                        steps = []

                        def wo_mm(qq):
                            q = j * 4 + qq
                            if qq % 2 == 1:
                                pwA = pvp.tile([128, QB], f32, tag="pv",
                                               name=f"pwA_{q}")
                                pwB = pvp.tile([128, 256], f32, tag="pv",
                                               name=f"pwB_{q}")
                            else:
                                pw = wpp.tile([128, 1024], f32, tag="wp",
                                              name=f"pw_{q}")
                                pwA, pwB = pw[:, 0:QB], pw[:, QB:C]
                            pws[qq] = (pwA, pwB)
                            for dst, (n0, n1) in ((pwA, (0, 512)),
                                                  (pwB, (512, 768))):
                                nc.tensor.matmul(
                                    dst,
                                    lhsT=outt01[:, q * 128:(q + 1) * 128],
                                    rhs=wo01[:, n0:n1],
                                    start=True, stop=False)
                                nc.tensor.matmul(
                                    dst,
                                    lhsT=outt2[:, q * 128:(q + 1) * 128],
                                    rhs=wo2[:, n0:n1],
                                    start=False, stop=True)

                        def wo_out(qq):
                            q = j * 4 + qq
                            pwA, pwB = pws[qq]
                            ot = osp.tile([128, C], bf16, tag="ot",
                                          name=f"ot_{q}")
                            cpf = (nc.scalar.copy if last
                                   else nc.vector.tensor_copy)
                            cpf(ot[:, 0:QB], pwA)
                            cpf(ot[:, QB:C], pwB)
                            nc.sync.dma_start(
                                out=out_p[q * 128:(q + 1) * 128, :], in_=ot)
                        for qq in range(4):
                            steps.append(lambda qq=qq: wo_mm(qq))
                            steps.append(lambda qq=qq: wo_out(qq))
                        return steps

                    def p_steps(unit, expt, wo_js=(), last=False):
                        """PV matmul chunk-steps, copy+normalize, then the
                        output projections listed in wo_js."""
                        hh, j = unit
                        nch = 4 * (j + 1)
                        heads = [(0, 0), (1, 1)] if hh == "01" else [(2, 0)]
                        pos = {}
                        steps = []

                        def setup():
                            for h, _ in heads:
                                pos[h] = pvp.tile([128, QB], f32, tag="pv",
                                                  name=f"po_{h}_{j}")

                        def chunk(c):
                            u = c - 4 * j
                            off = KC * u if u > 0 else 0
                            for h, hh_slot in heads:
                                nc.tensor.matmul(
                                    pos[h][0:DH + 1, off:QB],
                                    lhsT=vones[:, c, h, :],
                                    rhs=expt[:, hh_slot, c, off:QB],
                                    start=(c == 0), stop=(c == nch - 1),
                                    skip_group_check=True)

                        steps.append(setup)
                        for c0 in range(0, nch, 2):
                            def two(c0=c0):
                                chunk(c0)
                                chunk(c0 + 1)
                            steps.append(two)

                        def fin(h, hh_slot):
                            po = pos[h]
                            nc.vector.tensor_copy(
                                tgt_of(h)[:, j * QB:(j + 1) * QB], po[0:DH, :])
                            nc.vector.tensor_copy(
                                denom[0:1, h * QB:(h + 1) * QB],
                                po[DH:DH + 1, :])

                        def norm_unit():
                            usl = slice(heads[0][0] * QB,
                                        (heads[-1][0] + 1) * QB)
                            with nc.allow_low_precision(reason="softmax denom reciprocal: 18-bit approx"):
                                nc.vector.reciprocal_approx_fast(
                                    out=recd[0:1, usl], in_=denom[0:1, usl])
                            if last:
                                # scores done: broadcast 1/denom across
                                # partitions via a K=1 matmul into a free sc
                                # psum slot instead of the DRAM round-trip
                                for h, _ in heads:
                                    bctp = scp.tile([128, QB], f32, tag="sc",
                                                    name=f"bctp_{h}_{j}")
                                    nc.tensor.matmul(
                                        bctp[0:64, :], lhsT=ones64f[0:1, :],
                                        rhs=recd[0:1, h * QB:(h + 1) * QB],
                                        start=True, stop=True)
                                    tgt = tgt_of(h)
                                    sl = slice(j * QB, (j + 1) * QB)
                                    nc.vector.tensor_mul(
                                        tgt[:, sl], tgt[:, sl], bctp[0:64, :])
                                return
                            nc.sync.dma_start(out=recd_d[0:1, usl],
                                              in_=recd[0:1, usl])
                            for h, _ in heads:
                                base = 64 if h == 1 else 0
                                src = recd_d[0:1, h * QB:(h + 1) * QB]
                                bsrc = bass.AP(
                                    tensor=src.tensor, offset=src.offset,
                                    ap=[[0, 64]] + list(src.ap[1:]))
                                bct = bcp.tile([128, QB], f32, tag="bct",
                                               name=f"bct_{h}_{j}")
                                nc.sync.dma_start(
                                    out=bct[base:base + 64, :], in_=bsrc)
                                tgt = tgt_of(h)
                                sl = slice(j * QB, (j + 1) * QB)
                                nc.vector.tensor_mul(
                                    tgt[:, sl], tgt[:, sl],
                                    bct[base:base + 64, :])

                        for h, hh_slot in heads:
                            steps.append(lambda h=h, s=hh_slot: fin(h, s))
                        steps.append(norm_unit)
                        for wj in wo_js:
                            steps = steps + wo_steps(wj, last=last)
                        return steps

                    def vproj_late(t):
                        vproj(t, False, wpp, tag="wp")

                    def proj_h1(m):
                        """h1-half projection as attention filler: psum from
                        the wo pool, copies on DVE."""
                        ps = wpp.tile([128, 1024], f32, tag="wp",
                                      name=f"pph1_{m}")
                        emit_proj(m, 1, [ps[:, 0:QB], ps[:, QB:2 * QB]],
                                  on_act=False)

                    units = [("01", 0), ("01", 1), ("2", 0), ("2", 1),
                             ("01", 2), ("2", 2), ("01", 3), ("2", 3)]
                    # wo(j) placement: attached to unit index -> executes
                    # under the following unit (which has exp slack)
                    wo_of = {3: [0], 5: [1, 2], 7: [3]}
                    # filler work appended to unit i's p-list (executes under
                    # unit i+1)
                    fillers = {
                        0: [lambda t=t: vproj_late(t) for t in range(4, 8)]
                           + [lambda: proj_h1(0)],
                        2: [lambda t=t: vproj_late(t) for t in range(8, 12)]
                           + [lambda: proj_h1(1)],
                        3: [lambda: proj_h1(2)],
                        4: [lambda t=t: vproj_late(t) for t in range(12, 16)],
                    }

                    # lag-1 pipeline, interleaved at step granularity: PE runs
                    # the previous unit's PV/Wo steps in the gaps between this
                    # unit's score groups (which are paced by ACT's exp).
                    prev_p = []
                    for i, u in enumerate(units):
                        last = i == len(units) - 1
                        S = s_steps(u, expts[i % 2])
                        if last:
                            # fold the final unit's own PV steps behind its
                            # score groups (lag 2)
                            own = p_steps(u, expts[i % 2],
                                          wo_js=wo_of.get(i, ()), last=True)
                        done = 0
                        own_done = 0
                        for gi, s in enumerate(S):
                            s()
                            want = ((gi + 1) * len(prev_p)) // len(S)
                            while done < want:
                                prev_p[done]()
                                done += 1
                            if last and gi >= 2:
                                while own_done < min(gi - 1, len(S) - 1) + 1:
                                    own[own_done]()
                                    own_done += 1
                        while done < len(prev_p):
                            prev_p[done]()
                            done += 1
                        if last:
                            prev_p = own[own_done:]
                        else:
                            prev_p = p_steps(u, expts[i % 2],
                                             wo_js=wo_of.get(i, ()))
                            prev_p = prev_p + fillers.get(i, [])
                    for p in prev_p:
                        p()

            rp_cm.__exit__(None, None, None)

    nc.compile()
    return nc


def _host_prep(x, Wqkv, Wo, seq_len):
    import ml_dtypes
    bf16 = ml_dtypes.bfloat16
    x = np.asarray(x, dtype=np.float32)
    Wqkv = np.asarray(Wqkv, dtype=np.float32)
    Wo = np.asarray(Wo, dtype=np.float32)
    off = int(np.asarray(seq_len).reshape(()))

    inv = 1.0 / (10000.0 ** (np.arange(0, DH, 2, dtype=np.float64) / DH))  # [32]
    pos = np.arange(T, dtype=np.float64) + off
    ang = pos[:, None] * inv[None, :]                 # [T, 32]
    cs = np.cos(ang).T                                # [32, T]
    sn = np.sin(ang).T
    cos128 = np.empty((128, T), np.float32)
    sin128 = np.empty((128, T), np.float32)
    for blk in range(2):
        r0 = blk * 64
        cos128[r0:r0 + 32] = cs
        cos128[r0 + 32:r0 + 64] = cs
        # row-swapped + sign-folded: row s holds the coefficient X[s] is
        # multiplied by when producing output row s^32 (see rope()).
        sin128[r0:r0 + 32] = sn
        sin128[r0 + 32:r0 + 64] = -sn

    in_maps = []
    for core in range(NC_):
        b, g = core // 4, core % 4
        hs = [3 * g, 3 * g + 1, 3 * g + 2]
        q = [Wqkv[:, h * DH:(h + 1) * DH] for h in hs]
        k = [Wqkv[:, C + h * DH:C + (h + 1) * DH] for h in hs]
        v = [Wqkv[:, 2 * C + h * DH:2 * C + (h + 1) * DH] for h in hs]
        wqkv_l = np.concatenate(
            [q[0], q[1], k[0], k[1], q[2], k[2], v[0], v[1], v[2]], axis=1)
        in_maps.append({
            "xT": np.ascontiguousarray(x[b].T).astype(bf16),
            "wqkv": np.ascontiguousarray(wqkv_l).astype(bf16),
            "wo": np.ascontiguousarray(
                Wo[g * HPC * DH:(g + 1) * HPC * DH, :]).astype(bf16),
            "cosT": cos128.astype(bf16),
            "sinT": sin128.astype(bf16),
        })
    return in_maps


def _run(in_maps, trace=False):
    global _prog
    from concourse.bass_utils import run_bass_kernel_spmd
    if _prog is None:
        _prog = _build()
    return run_bass_kernel_spmd(_prog, in_maps, list(range(NC_)), trace=trace)


def kernel(x, Wqkv, Wo, seq_len):
    in_maps = _host_prep(x, Wqkv, Wo, seq_len)
    res = _run(in_maps, trace=False)
    out = np.zeros((B, T, C), dtype=np.float32)
    for core in range(NC_):
        out[core // 4] += res.results[core]["out"].astype(np.float32)
    return out
